# revision 1
# baseline (speedup 1.0000x reference)
"""Trainium2 Bass kernel for nn_CrossAttention (B=2, L=1024, S=2048, DIM=1024, H=16 heads).

Sharding: tensor-parallel over heads x data-parallel over batch.
Core c handles batch b = c//4 and head-group g = c%4 (4 heads = 256 of the
1024 hidden channels).  Each core computes, for its (b, g):

    QT = (Wq_g)^T x_q^T          [256, 1024]   (m on partitions)
    KT = (Wk_g)^T x_k^T          [256, 2048]
    V  = x_v Wv_g                [2048, 256]   (s on partitions)
    per head h (d=64):
        ST_h = KT_h^T' ...       S^T[s, l] = k_s . q_l   (s on partitions)
        P_h  = exp(SCALE * ST_h)            (unnormalized, s on partitions)
        [O^T_h ; sums_h] = [V_h | 1]^T @ P_h   (ones-column folds the softmax
                                                denominator into the matmul)
        XgT_h = O^T_h * (1/sums_h)          (gpsimd partition broadcast)
    out_partial = XgT^T @ Wo_g   [1024, 1024]

Host gathers: out[b] = sum_g out_partial[4b+g] + bo.

The kernel is scalar(exp)-bound in its core, so the structure minimizes
time-to-first-exp and keeps the 64-exp stream dense:
  A: xq/xk land as ONE strided DMA each (full 16-engine striping, no
     per-chunk issue pacing).  Projections run as 2-bank double passes
     (Q-mt0, K-sh0 both mt, then the first 8 ST+exp steps standalone while
     K-sh1 passes run).  PSUM->SBUF casts on the vector engine, off the
     scalar stream.
  B: V projection (2-bank accumulators, double-pass over cached xv chunks)
     interleaved with the remaining 24 lch0 ST+exp steps.
  C: O(lch0) + ST/exp(lch1).
  D: O(lch1) (PSUM banks from the shared ring, no false deps on C's tail)
     + Wo(lch0) interleaved.   E: Wo(lch1).
Output partials are written bf16 (host accumulates fp32); the softmax
reciprocal uses the fast approximate DVE op + gpsimd partition broadcast.
"""

import sys

if "/opt/trn_rl_repo" not in sys.path:
    sys.path.insert(0, "/opt/trn_rl_repo")

import numpy as np

B, L, S, C = 2, 1024, 2048, 1024
NH, D = 16, 64          # total heads, head dim
HPC = 4                 # heads per core
M = HPC * D             # 256 output channels per core
SCALE = D ** -0.5
P = 128                 # partitions
NCORES = 8
CK = C // P             # 8 c-tiles
NST = S // P            # 16 s-tiles
LCH = 512               # l-chunk
NLCH = L // LCH         # 2

_cache = {}


def _build(debug_dumps=False):
    import concourse.tile as tile
    from concourse import mybir, bacc

    f32 = mybir.dt.float32
    bf16 = mybir.dt.bfloat16

    nc = bacc.Bacc("TRN2", target_bir_lowering=False, debug=False)

    xqT = nc.dram_tensor("xqT", [C, L], bf16, kind="ExternalInput")
    xkT = nc.dram_tensor("xkT", [C, S], bf16, kind="ExternalInput")
    xvT = nc.dram_tensor("xvT", [C, S], bf16, kind="ExternalInput")
    wq = nc.dram_tensor("wq", [C, M], bf16, kind="ExternalInput")
    wk = nc.dram_tensor("wk", [C, M], bf16, kind="ExternalInput")
    wv = nc.dram_tensor("wv", [C, M], bf16, kind="ExternalInput")
    wo = nc.dram_tensor("wo", [M, C], bf16, kind="ExternalInput")
    outp = nc.dram_tensor("outp", [L, C], bf16, kind="ExternalOutput")
    if debug_dumps:
        dbg_qt = nc.dram_tensor("dbg_qt", [P, 2, L], bf16, kind="ExternalOutput")
        dbg_kt = nc.dram_tensor("dbg_kt", [P, 2, S], bf16, kind="ExternalOutput")
        dbg_vones = nc.dram_tensor("dbg_vones", [P, NST, HPC, D + 1], bf16,
                                   kind="ExternalOutput")
        dbg_pt = nc.dram_tensor("dbg_pt", [P, 2, LCH], bf16, kind="ExternalOutput")
        dbg_rc = nc.dram_tensor("dbg_rc", [1, LCH], f32, kind="ExternalOutput")
        dbg_bc = nc.dram_tensor("dbg_bc", [D, LCH], f32, kind="ExternalOutput")
        dbg_xgt = nc.dram_tensor("dbg_xgt", [P, 2, L], bf16, kind="ExternalOutput")

    with tile.TileContext(nc) as tc:
        with tc.tile_pool(name="singles", bufs=1) as singles, \
             tc.tile_pool(name="xv_pool", bufs=2) as xvp, \
             tc.tile_pool(name="pts", bufs=34) as pts, \
             tc.tile_pool(name="small", bufs=4) as small, \
             tc.tile_pool(name="obuf", bufs=10) as obuf:

            # ---- persistent SBUF ----
            wq_sb = singles.tile([P, CK, M], bf16, tag="wq")
            wk_sb = singles.tile([P, CK, M], bf16, tag="wk")
            wv_sb = singles.tile([P, CK, M], bf16, tag="wv")
            wo_sb = singles.tile([P, M // P, C], bf16, tag="wo")
            xq_sb = singles.tile([P, CK, L], bf16, tag="xq")
            xk_sb = singles.tile([P, CK, S], bf16, tag="xkc")
            SH = S // 2
            # single big strided DMAs: 16-engine striping from issue #1,
            # no per-chunk sync-queue issue pacing.
            nc.sync.dma_start(wq_sb[:], wq.rearrange("(ck p) m -> p ck m", p=P))
            HCK = CK // 2
            for h in range(2):
                nc.sync.dma_start(
                    xq_sb[:, h * HCK:(h + 1) * HCK, :],
                    xqT[h * HCK * P:(h + 1) * HCK * P, :]
                    .rearrange("(ck p) l -> p ck l", p=P))
            nc.sync.dma_start(wk_sb[:], wk.rearrange("(ck p) m -> p ck m", p=P))
            for h in range(2):
                nc.sync.dma_start(
                    xk_sb[:, h * HCK:(h + 1) * HCK, 0:SH],
                    xkT[h * HCK * P:(h + 1) * HCK * P, 0:SH]
                    .rearrange("(ck p) s -> p ck s", p=P))
            nc.sync.dma_start(xk_sb[:, :, SH:S],
                              xkT[:, SH:S].rearrange("(ck p) s -> p ck s", p=P))
            nc.sync.dma_start(wv_sb[:], wv.rearrange("(ck p) m -> p ck m", p=P))
            nc.sync.dma_start(wo_sb[:], wo.rearrange("(kt p) n -> p kt n", p=P))

            kt_sb = singles.tile([P, 2, S], bf16, tag="kt")        # [m%128, m//128, s]
            qt_sb = singles.tile([P, 2, L], bf16, tag="qt")        # [m%128, m//128, l]
            vones = singles.tile([P, NST, HPC, D + 1], bf16, tag="vones")
            xgt_sb = singles.tile([P, 2, L], bf16, tag="xgt")
            stage = singles.tile([P, D], f32, tag="stage")
            nc.vector.memset(stage[:], 1.0)
            nc.vector.tensor_copy(vones[:, :, :, D:D + 1],
                                  stage[:].rearrange("p (a b) -> p a b", a=NST)[:, :, :, None])

            # ---- step helpers ----
            def st_step(lch, pair, st):
                """ST pair matmuls + exp; returns the PT tile."""
                lsl = slice(lch * LCH, (lch + 1) * LCH)
                ssl = slice(st * P, (st + 1) * P)
                st_ps = pst.tile([P, 2, LCH], f32, tag="st", name=f"stps_{lch}_{pair}_{st}")
                nc.tensor.matmul(
                    st_ps[:, 0, :], kt_sb[0:D, pair, ssl], qt_sb[0:D, pair, lsl],
                    start=True, stop=True)
                nc.tensor.matmul(
                    st_ps[:, 1, :], kt_sb[D:P, pair, ssl], qt_sb[D:P, pair, lsl],
                    start=True, stop=True, tile_position=(64, 0))
                pt_t = pts.tile([P, 2, LCH], bf16, tag="pt", name=f"pt_{lch}_{pair}_{st}")
                nc.scalar.activation(pt_t[:], st_ps[:],
                                     mybir.ActivationFunctionType.Exp, scale=SCALE)
                if debug_dumps and lch == 0 and pair == 0 and st == 0:
                    nc.sync.dma_start(dbg_pt[:], pt_t[:])
                return pt_t

            def o_step(o_ps, lch, pair, st, pt_t):
                for hh in range(2):
                    nc.tensor.matmul(
                        o_ps[hh][:], vones[:, st, pair * 2 + hh, :], pt_t[:, hh, :],
                        start=(st == 0), stop=(st == NST - 1))

            def norm_pair(lch, pair, o_ps):
                """fast reciprocal of sums row -> gpsimd partition broadcast
                -> normalized XgT (no PSUM bank, no tensor-engine matmul)."""
                lsl = slice(lch * LCH, (lch + 1) * LCH)
                for hh in range(2):
                    # rc lives at partition 0: the gpsimd broadcast firmware
                    # reads the source on Q7 core 0, which only sees
                    # partitions 0-15.  Stage the PSUM sums row into SBUF
                    # first (custom-DVE bit ops need an SBUF source).
                    sums_sb = small.tile([1, LCH], f32, tag="sums")
                    nc.vector.tensor_copy(sums_sb[:], o_ps[hh][D:D + 1, :])
                    rc = small.tile([1, LCH], f32, tag="rc")
                    nc.vector.reciprocal_approx_fast(rc[:], sums_sb[:])
                    bc_sb = small.tile([D, LCH], f32, tag="bc")
                    nc.gpsimd.partition_broadcast(bc_sb[:], rc[:])
                    if debug_dumps and lch == 0 and pair == 0 and hh == 0:
                        nc.sync.dma_start(dbg_rc[:], rc[:])
                        nc.sync.dma_start(dbg_bc[:], bc_sb[:])
                    nc.vector.tensor_mul(
                        xgt_sb[hh * D:(hh + 1) * D, pair, lsl],
                        o_ps[hh][0:D, :], bc_sb[:])

            def wo_step(pool, lt, nch, cast_eng):
                wo_ps = pool.tile([P, 512], f32, tag="wo", name=f"wops_{lt}_{nch}")
                for kt in range(2):
                    nc.tensor.matmul(
                        wo_ps[:], xgt_sb[:, kt, lt * P:(lt + 1) * P],
                        wo_sb[:, kt, nch * 512:(nch + 1) * 512],
                        start=(kt == 0), stop=(kt == 1))
                ob_sb = obuf.tile([P, 512], bf16, tag="ob")
                if cast_eng == "scalar":
                    nc.scalar.copy(ob_sb[:], wo_ps[:])
                    nc.scalar.dma_start(
                        outp[lt * P:(lt + 1) * P, nch * 512:(nch + 1) * 512], ob_sb[:])
                else:
                    nc.vector.tensor_copy(ob_sb[:], wo_ps[:])
                    nc.gpsimd.dma_start(
                        outp[lt * P:(lt + 1) * P, nch * 512:(nch + 1) * 512], ob_sb[:])

            # ---- PSUM pool timeline (LIFO):
            #   shared(2) > pst(4) > [psp(2) A] > [ps1(2) C] > close pst >
            #   [wo(4) D/E] > close shared
            shared_cm = tc.tile_pool(name="ps_shared", bufs=2, space="PSUM")
            shared = shared_cm.__enter__()
            pst_cm = tc.tile_pool(name="ps_st", bufs=2, space="PSUM")
            pst = pst_cm.__enter__()

            pt0 = {}   # (pair, st) -> PT tile for lch 0
            pt1 = {}

            # PE p-state warm-up: throwaway matmuls on a memset tile while
            # the input DMAs land, so the projections start at full clock
            # instead of the 0.65-1.2 GHz cold states.  Output is unread.
            warm_sb = singles.tile([P, 512], bf16, tag="warm")
            nc.vector.memset(warm_sb[:], 1.0)
            with tc.tile_pool(name="ps_warm", bufs=1, space="PSUM") as psw:
                warm = psw.tile([P, 512], f32, tag="warm")
                for i in range(10):
                    nc.tensor.matmul(
                        warm[:], warm_sb[:, 0:128], warm_sb[:],
                        start=True, stop=True)

            # =========== Phase A: QT + KT projections ===========
            with tc.tile_pool(name="ps_proj", bufs=2, space="PSUM") as psp:

                def q_pass(mt, st_jobs=()):
                    st_jobs = list(st_jobs)
                    q_ps = [psp.tile([P, 512], f32, tag="pp", name=f"qtps{mt}_{lh}")
                            for lh in range(2)]
                    for ck in range(CK):
                        for lh in range(2):
                            nc.tensor.matmul(
                                q_ps[lh][:],
                                wq_sb[:, ck, mt * P:(mt + 1) * P],
                                xq_sb[:, ck, lh * 512:(lh + 1) * 512],
                                start=(ck == 0), stop=(ck == CK - 1))
                        if st_jobs and ck % 2 == 1:
                            pair, st = st_jobs.pop(0)
                            pt0[(pair, st)] = st_step(0, pair, st)
                    for lh in range(2):
                        nc.vector.tensor_copy(
                            qt_sb[:, mt, lh * 512:(lh + 1) * 512], q_ps[lh][:])

                def k_pass(mt, sh, st_jobs=()):
                    st_jobs = list(st_jobs)
                    k_ps = [psp.tile([P, 512], f32, tag="pp",
                                     name=f"ktps{sh}_{mt}_{nh}") for nh in range(2)]
                    for ck in range(CK):
                        for nh in range(2):
                            nc.tensor.matmul(
                                k_ps[nh][:],
                                wk_sb[:, ck, mt * P:(mt + 1) * P],
                                xk_sb[:, ck, sh * SH + nh * 512:sh * SH + (nh + 1) * 512],
                                start=(ck == 0), stop=(ck == CK - 1))
                        if st_jobs and ck % 2 == 1:
                            pair, st = st_jobs.pop(0)
                            pt0[(pair, st)] = st_step(0, pair, st)
                    for nh in range(2):
                        nc.vector.tensor_copy(
                            kt_sb[:, mt, sh * SH + nh * 512:sh * SH + (nh + 1) * 512],
                            k_ps[nh][:])

                q_pass(0)
                k_pass(0, 0)
                # first 8 ST+exp steps run standalone: they only need the
                # mt0 projections and KT s-half 0 (s-tiles 0-7).
                for st in range(8):
                    pt0[(0, st)] = st_step(0, 0, st)
                k_pass(0, 1)
                q_pass(1, [(0, st) for st in range(8, 12)])
                k_pass(1, 0, [(0, st) for st in range(12, 16)])
                k_pass(1, 1, [(1, st) for st in range(4)])

            # =========== Phase B: V projection + remaining lch0 ST steps ====
            b_jobs = [(1, st) for st in range(4, 16)]
            for q in range(4):
                xv_t = xvp.tile([P, CK, 4 * P], bf16, tag="xv")
                nc.sync.dma_start(
                    xv_t[:], xvT[:, q * 4 * P:(q + 1) * 4 * P]
                    .rearrange("(ck p) s -> p ck s", p=P))
                for half in range(2):
                    v_ps = [shared.tile([P, M], f32, tag="sh",
                                        name=f"vps{q}_{half}_{i}")
                            for i in range(2)]
                    for ck in range(CK):
                        for st2 in range(2):
                            st4 = half * 2 + st2
                            nc.tensor.matmul(
                                v_ps[st2][:],
                                xv_t[:, ck, st4 * P:(st4 + 1) * P],
                                wv_sb[:, ck, :],
                                start=(ck == 0), stop=(ck == CK - 1))
                        if ck % 2 == half and ck < 6 and b_jobs:
                            pair, st = b_jobs.pop(0)
                            pt0[(pair, st)] = st_step(0, pair, st)
                    for st2 in range(2):
                        st = q * 4 + half * 2 + st2
                        nc.vector.tensor_copy(
                            vones[:, st, :, 0:D],
                            v_ps[st2][:].rearrange("p (h d) -> p h d", h=HPC))

            # =========== Phase C: O(lch0) + lch1 ST/exp ===========
            with tc.tile_pool(name="ps_c", bufs=2, space="PSUM") as ps1:
                for pair in range(2):
                    o_ps = [ps1.tile([D + 1, LCH], f32, tag="ps1",
                                     name=f"ops0_{pair}_{i}") for i in range(2)]
                    for st in range(NST):
                        o_step(o_ps, 0, pair, st, pt0.pop((pair, st)))
                        pt1[(pair, st)] = st_step(1, pair, st)
                    norm_pair(0, pair, o_ps)

            pst_cm.__exit__(None, None, None)

            # =========== Phases D+E: O(lch1) + Wo ===========
            with tc.tile_pool(name="ps_wo", bufs=6, space="PSUM") as pswo:
                wo_jobs0 = [(lt, nch) for lt in range(4) for nch in range(2)]
                ncast = 0
                for pair in range(2):
                    o_ps = [shared.tile([D + 1, LCH], f32, tag="sh",
                                        name=f"ops1_{pair}_{i}") for i in range(2)]
                    for st in range(NST):
                        o_step(o_ps, 1, pair, st, pt1.pop((pair, st)))
                        if st % 4 == 1 and wo_jobs0:
                            wo_step(pswo, *wo_jobs0.pop(0),
                                    "scalar" if ncast % 2 == 0 else "vector")
                            ncast += 1
                    norm_pair(1, pair, o_ps)
                for lt, nch in wo_jobs0:
                    wo_step(pswo, lt, nch, "scalar" if ncast % 2 == 0 else "vector")
                    ncast += 1

                for lt in range(4, 8):
                    for nch in range(2):
                        wo_step(pswo, lt, nch,
                                "scalar" if ncast % 2 == 0 else "vector")
                        ncast += 1

                if debug_dumps:
                    nc.sync.dma_start(dbg_qt[:], qt_sb[:])
                    nc.sync.dma_start(dbg_kt[:], kt_sb[:])
                    nc.sync.dma_start(dbg_vones[:], vones[:])
                    nc.sync.dma_start(dbg_xgt[:], xgt_sb[:])

            shared_cm.__exit__(None, None, None)

    nc.compile()
    return nc


def _get_nc():
    if "nc" not in _cache:
        _cache["nc"] = _build()
    return _cache["nc"]


def _make_in_maps(inputs):
    import ml_dtypes

    bf16 = ml_dtypes.bfloat16
    query = np.asarray(inputs["query"], dtype=np.float32)
    key = np.asarray(inputs["key"], dtype=np.float32)
    value = np.asarray(inputs["value"], dtype=np.float32)
    Wq = np.asarray(inputs["Wq"], dtype=np.float32)
    Wk = np.asarray(inputs["Wk"], dtype=np.float32)
    Wv = np.asarray(inputs["Wv"], dtype=np.float32)
    Wo = np.asarray(inputs["Wo"], dtype=np.float32)

    qT = [np.ascontiguousarray(query[b].T).astype(bf16) for b in range(B)]
    kT = [np.ascontiguousarray(key[b].T).astype(bf16) for b in range(B)]
    vT = [np.ascontiguousarray(value[b].T).astype(bf16) for b in range(B)]
    wq_s = [np.ascontiguousarray(Wq[:, g * M:(g + 1) * M]).astype(bf16) for g in range(4)]
    wk_s = [np.ascontiguousarray(Wk[:, g * M:(g + 1) * M]).astype(bf16) for g in range(4)]
    wv_s = [np.ascontiguousarray(Wv[:, g * M:(g + 1) * M]).astype(bf16) for g in range(4)]
    wo_s = [np.ascontiguousarray(Wo[g * M:(g + 1) * M, :]).astype(bf16) for g in range(4)]

    in_maps = []
    for core in range(NCORES):
        b, g = core // 4, core % 4
        in_maps.append({
            "xqT": qT[b], "xkT": kT[b], "xvT": vT[b],
            "wq": wq_s[g], "wk": wk_s[g], "wv": wv_s[g], "wo": wo_s[g],
        })
    return in_maps


def kernel(query, key, value, Wq, Wk, Wv, Wo, bo):
    from concourse.bass_utils import run_bass_kernel_spmd

    nc = _get_nc()
    bo = np.asarray(bo, dtype=np.float32)
    in_maps = _make_in_maps(dict(query=query, key=key, value=value,
                                 Wq=Wq, Wk=Wk, Wv=Wv, Wo=Wo))

    res = run_bass_kernel_spmd(nc, in_maps, core_ids=list(range(NCORES)))

    out = np.zeros((B, L, C), dtype=np.float32)
    for core in range(NCORES):
        b = core // 4
        out[b] += np.asarray(res.results[core]["outp"], dtype=np.float32)
    out += bo[None, None, :]
    return out



# revision 10
# speedup vs baseline: 1.0004x; 1.0004x over previous
"""Trainium2 Bass kernel for nn_CrossAttention (B=2, L=1024, S=2048, DIM=1024, H=16 heads).

Sharding: tensor-parallel over heads x data-parallel over batch.
Core c handles batch b = c//4 and head-group g = c%4 (4 heads = 256 of the
1024 hidden channels).  Each core computes, for its (b, g):

    QT = (Wq_g)^T x_q^T          [256, 1024]   (m on partitions)
    KT = (Wk_g)^T x_k^T          [256, 2048]
    V  = x_v Wv_g                [2048, 256]   (s on partitions)
    per head h (d=64):
        ST_h = KT_h^T' ...       S^T[s, l] = k_s . q_l   (s on partitions)
        P_h  = exp(SCALE * ST_h)            (unnormalized, s on partitions)
        [O^T_h ; sums_h] = [V_h | 1]^T @ P_h   (ones-column folds the softmax
                                                denominator into the matmul)
        XgT_h = O^T_h * (1/sums_h)          (gpsimd partition broadcast)
    out_partial = XgT^T @ Wo_g   [1024, 1024]

Host gathers: out[b] = sum_g out_partial[4b+g] + bo.

v2 schedule: the kernel is jointly PE- and ACT(exp)-bound; the structure
minimizes time-to-first-exp, keeps the 64-exp stream dense, and minimizes
the serial tail after the last exp:
  - Input DMAs are split and priority-ordered (wq, xq/2, wk, xk/4 s-slices,
    wv, xv/4, wo) so the first ST+exp fires as soon as 4MB has landed.
  - Phase A consumes DMA chunks as they arrive (ck-major, both m-tiles per
    ck) and fires all pair-0 ST steps; phase B (V projection) paces the
    pair-1 ST steps.  PE never idles >3us, so the HAM clock stays at 2.4GHz.
  - Merged phase: per slot (pair, st): ST(lch1) matmuls + O(lch0) + O(lch1,
    2-slot lag).  Both O accumulations for a pair finish with the exp
    stream, so the post-exp tail is only norm + Wo (~8us) instead of the
    whole O(lch1)+Wo pipeline (~21us).
  - Norms are split per (pair, lch): Wo jobs for lch0 start while lch1's
    norm still runs.  PSUM->SBUF casts alternate scalar/vector engines.
Output partials are written bf16 (host accumulates fp32); the softmax
reciprocal uses the fast approximate DVE op + gpsimd partition broadcast.
"""

import sys

if "/opt/trn_rl_repo" not in sys.path:
    sys.path.insert(0, "/opt/trn_rl_repo")

import numpy as np

B, L, S, C = 2, 1024, 2048, 1024
NH, D = 16, 64          # total heads, head dim
HPC = 4                 # heads per core
M = HPC * D             # 256 output channels per core
SCALE = D ** -0.5
P = 128                 # partitions
NCORES = 8
CK = C // P             # 8 c-tiles
NST = S // P            # 16 s-tiles
LCH = 512               # l-chunk
NLCH = L // LCH         # 2

_cache = {}


def _build():
    import concourse.tile as tile
    from concourse import mybir, bacc

    f32 = mybir.dt.float32
    bf16 = mybir.dt.bfloat16

    nc = bacc.Bacc("TRN2", target_bir_lowering=False, debug=False)

    xqT = nc.dram_tensor("xqT", [C, L], bf16, kind="ExternalInput")
    xkT = nc.dram_tensor("xkT", [C, S], bf16, kind="ExternalInput")
    xvT = nc.dram_tensor("xvT", [C, S], bf16, kind="ExternalInput")
    wq = nc.dram_tensor("wq", [C, M], bf16, kind="ExternalInput")
    wk = nc.dram_tensor("wk", [C, M], bf16, kind="ExternalInput")
    wv = nc.dram_tensor("wv", [C, M], bf16, kind="ExternalInput")
    wo = nc.dram_tensor("wo", [M, C], bf16, kind="ExternalInput")
    outp = nc.dram_tensor("outp", [L, C], bf16, kind="ExternalOutput")

    with tile.TileContext(nc) as tc:
        with tc.tile_pool(name="singles", bufs=1) as singles, \
             tc.tile_pool(name="xk_pool", bufs=3) as xkp, \
             tc.tile_pool(name="xv_pool", bufs=3) as xvp, \
             tc.tile_pool(name="pts", bufs=36) as pts, \
             tc.tile_pool(name="small", bufs=3) as small, \
             tc.tile_pool(name="obuf", bufs=4) as obuf:

            # ---- persistent SBUF ----
            wq_sb = singles.tile([P, CK, M], bf16, tag="wq")
            wk_sb = singles.tile([P, CK, M], bf16, tag="wk")
            wv_sb = singles.tile([P, CK, M], bf16, tag="wv")
            wo_sb = singles.tile([P, M // P, C], bf16, tag="wo")
            xq_sb = singles.tile([P, CK, L], bf16, tag="xq")

            # DMA priority order = consumption order.  Chunks sized ~0.5-1MB
            # keep 16-queue striping while letting compute start early.
            HCK = CK // 2
            nc.sync.dma_start(wq_sb[:], wq.rearrange("(ck p) m -> p ck m", p=P))
            for h in range(2):
                nc.sync.dma_start(
                    xq_sb[:, h * HCK:(h + 1) * HCK, :],
                    xqT[h * HCK * P:(h + 1) * HCK * P, :]
                    .rearrange("(ck p) l -> p ck l", p=P))
            nc.sync.dma_start(wk_sb[:], wk.rearrange("(ck p) m -> p ck m", p=P))
            # xk split along s (4 slices of 512, rotating pool) so ST steps
            # unlock in order and SBUF holds at most 3 slices.
            xk_t = []
            for ss in range(3):
                t = xkp.tile([P, CK, 512], bf16, tag="xk", name=f"xk{ss}")
                nc.sync.dma_start(
                    t[:], xkT[:, ss * 512:(ss + 1) * 512]
                    .rearrange("(ck p) s -> p ck s", p=P))
                xk_t.append(t)
            nc.sync.dma_start(wv_sb[:], wv.rearrange("(ck p) m -> p ck m", p=P))
            # 4th xk slice reuses slice-0's buffer (frees after K-pass s0)
            t = xkp.tile([P, CK, 512], bf16, tag="xk", name="xk3")
            nc.sync.dma_start(
                t[:], xkT[:, 3 * 512:4 * 512].rearrange("(ck p) s -> p ck s", p=P))
            xk_t.append(t)
            # xv prefetched (4 chunks, 3 rotating buffers)
            xv_t = []
            for q in range(4):
                t = xvp.tile([P, CK, 4 * P], bf16, tag="xv", name=f"xv{q}")
                nc.sync.dma_start(
                    t[:], xvT[:, q * 4 * P:(q + 1) * 4 * P]
                    .rearrange("(ck p) s -> p ck s", p=P))
                xv_t.append(t)
            nc.sync.dma_start(wo_sb[:], wo.rearrange("(kt p) n -> p kt n", p=P))

            kt_sb = singles.tile([P, 2, S], bf16, tag="kt")        # [m%128, m//128, s]
            qt_sb = singles.tile([P, 2, L], bf16, tag="qt")        # [m%128, m//128, l]
            vones = singles.tile([P, NST, HPC, D + 1], bf16, tag="vones")
            xgt_sb = singles.tile([P, 2, L], bf16, tag="xgt")
            stage = singles.tile([P, D], f32, tag="stage")
            nc.vector.memset(stage[:], 1.0)
            nc.vector.tensor_copy(vones[:, :, :, D:D + 1],
                                  stage[:].rearrange("p (a b) -> p a b", a=NST)[:, :, :, None])

            # ---- step helpers ----
            def st_step(lch, pair, st):
                """ST pair matmuls + exp; returns the PT tile."""
                lsl = slice(lch * LCH, (lch + 1) * LCH)
                ssl = slice(st * P, (st + 1) * P)
                st_ps = pst.tile([P, 2, LCH], f32, tag="st", name=f"stps_{lch}_{pair}_{st}")
                nc.tensor.matmul(
                    st_ps[:, 0, :], kt_sb[0:D, pair, ssl], qt_sb[0:D, pair, lsl],
                    start=True, stop=True)
                nc.tensor.matmul(
                    st_ps[:, 1, :], kt_sb[D:P, pair, ssl], qt_sb[D:P, pair, lsl],
                    start=True, stop=True, tile_position=(64, 0))
                pt_t = pts.tile([P, 2, LCH], bf16, tag="pt", name=f"pt_{lch}_{pair}_{st}")
                nc.scalar.activation(pt_t[:], st_ps[:],
                                     mybir.ActivationFunctionType.Exp, scale=SCALE)
                return pt_t

            def o_step(o_ps, st, pair, pt_t):
                for hh in range(2):
                    nc.tensor.matmul(
                        o_ps[hh][:], vones[:, st, pair * 2 + hh, :], pt_t[:, hh, :],
                        start=(st == 0), stop=(st == NST - 1))

            def norm_lch(pair, lch, o_ps, tail=False):
                """per-(pair,lch) softmax normalize: batched fast reciprocal
                of the two sums rows -> gpsimd partition broadcasts -> scaled
                XgT.  At the tail the hh1 sums copy rides the (now idle)
                scalar engine so the two copies overlap."""
                lsl = slice(lch * LCH, (lch + 1) * LCH)
                for hh in range(2):
                    sums_sb = small.tile([1, LCH], f32, tag="sums")
                    if tail and hh == 1:
                        nc.scalar.copy(sums_sb[:], o_ps[hh][D:D + 1, :])
                    else:
                        nc.vector.tensor_copy(sums_sb[:], o_ps[hh][D:D + 1, :])
                    rc = small.tile([1, LCH], f32, tag="rc")
                    nc.vector.reciprocal_approx_fast(rc[:], sums_sb[:])
                    bc_sb = small.tile([D, LCH], f32, tag="bc")
                    nc.gpsimd.partition_broadcast(bc_sb[:], rc[:])
                    nc.vector.tensor_mul(
                        xgt_sb[hh * D:(hh + 1) * D, pair, lsl],
                        o_ps[hh][0:D, :], bc_sb[:])

            def wo_step(pool, lt, nch, cast_eng):
                wo_ps = pool.tile([P, 512], f32, tag="wo", name=f"wops_{lt}_{nch}")
                for kt in range(2):
                    nc.tensor.matmul(
                        wo_ps[:], xgt_sb[:, kt, lt * P:(lt + 1) * P],
                        wo_sb[:, kt, nch * 512:(nch + 1) * 512],
                        start=(kt == 0), stop=(kt == 1))
                ob_sb = obuf.tile([P, 512], bf16, tag="ob")
                if cast_eng == "scalar":
                    nc.scalar.copy(ob_sb[:], wo_ps[:])
                    nc.scalar.dma_start(
                        outp[lt * P:(lt + 1) * P, nch * 512:(nch + 1) * 512], ob_sb[:])
                else:
                    nc.vector.tensor_copy(ob_sb[:], wo_ps[:])
                    nc.gpsimd.dma_start(
                        outp[lt * P:(lt + 1) * P, nch * 512:(nch + 1) * 512], ob_sb[:])

            # ---- PSUM pool timeline (LIFO):
            #   pst(4) > [psw(1) warmup] > [psp(4) A] > [psv(4) B] >
            #   [ps_o(4) merged] > close pst > [ps_wo(6) Wo tail]
            pst_cm = tc.tile_pool(name="ps_st", bufs=2, space="PSUM")
            pst = pst_cm.__enter__()

            pt0 = {}   # (pair, st) -> PT tile for lch 0
            pt1 = {}

            # PE p-state warm-up while the first DMAs land.
            warm_sb = singles.tile([P, 512], bf16, tag="warm")
            nc.vector.memset(warm_sb[:], 1.0)
            with tc.tile_pool(name="ps_warm", bufs=1, space="PSUM") as psw:
                warm = psw.tile([P, 512], f32, tag="warm")
                for i in range(4):
                    nc.tensor.matmul(
                        warm[:], warm_sb[:, 0:128], warm_sb[:],
                        start=True, stop=True)

            # =========== Phase A: QT + KT projections, pair-0 ST steps ======
            with tc.tile_pool(name="ps_proj", bufs=4, space="PSUM") as psp:
                # Q: ck-major over both m-tiles, consuming xq chunks in order
                q_ps = [psp.tile([P, 512], f32, tag="pp", name=f"qtps{mt}_{lh}")
                        for mt in range(2) for lh in range(2)]
                for ck in range(CK):
                    for mt in range(2):
                        for lh in range(2):
                            nc.tensor.matmul(
                                q_ps[mt * 2 + lh][:],
                                wq_sb[:, ck, mt * P:(mt + 1) * P],
                                xq_sb[:, ck, lh * 512:(lh + 1) * 512],
                                start=(ck == 0), stop=(ck == CK - 1))
                for mt in range(2):
                    for lh in range(2):
                        nc.vector.tensor_copy(
                            qt_sb[:, mt, lh * 512:(lh + 1) * 512],
                            q_ps[mt * 2 + lh][:])

                # K: s-slice-major (512 s each), both m-tiles per slice;
                # pair-0 ST steps fire as their s-tiles' KT lands.
                a_jobs = []  # (pair, st) fired in A
                for ss in range(4):
                    k_ps = [psp.tile([P, 512], f32, tag="pp", name=f"ktps{ss}_{mt}")
                            for mt in range(2)]
                    for ck in range(CK):
                        for mt in range(2):
                            nc.tensor.matmul(
                                k_ps[mt][:],
                                wk_sb[:, ck, mt * P:(mt + 1) * P],
                                xk_t[ss][:, ck, :],
                                start=(ck == 0), stop=(ck == CK - 1))
                        # interleave pair-0 ST steps from previous slices
                        if ss > 0 and ck in (2, 5) :
                            st = (ss - 1) * 4 + (1 if ck == 2 else 2)
                            pt0[(0, st)] = st_step(0, 0, st)
                            a_jobs.append((0, st))
                    for mt in range(2):
                        nc.vector.tensor_copy(
                            kt_sb[:, mt, ss * 512:(ss + 1) * 512], k_ps[mt][:])
                    if ss > 0:
                        st0 = (ss - 1) * 4
                        for st in (st0, st0 + 3):
                            pt0[(0, st)] = st_step(0, 0, st)
                            a_jobs.append((0, st))
                # last slice's pair-0 steps
                for st in range(12, 16):
                    pt0[(0, st)] = st_step(0, 0, st)
                    a_jobs.append((0, st))

            # =========== Phase B: V projection + pair-1 lch0 ST steps =======
            b_jobs = [(1, st) for st in range(16)]
            with tc.tile_pool(name="ps_v", bufs=4, space="PSUM") as psv:
                for q in range(4):
                    for half in range(2):
                        v_ps = [psv.tile([P, M], f32, tag="vv",
                                         name=f"vps{q}_{half}_{i}")
                                for i in range(2)]
                        for ck in range(CK):
                            for st2 in range(2):
                                st4 = half * 2 + st2
                                nc.tensor.matmul(
                                    v_ps[st2][:],
                                    xv_t[q][:, ck, st4 * P:(st4 + 1) * P],
                                    wv_sb[:, ck, :],
                                    start=(ck == 0), stop=(ck == CK - 1))
                            if ck in (3, 7) and b_jobs:
                                pair, st = b_jobs.pop(0)
                                pt0[(pair, st)] = st_step(0, pair, st)
                        for st2 in range(2):
                            st = q * 4 + half * 2 + st2
                            nc.vector.tensor_copy(
                                vones[:, st, :, 0:D],
                                v_ps[st2][:].rearrange("p (h d) -> p h d", h=HPC))
                for pair, st in b_jobs:
                    pt0[(pair, st)] = st_step(0, pair, st)

            # ====== Merged phase: per slot: ST(lch1) + O(lch0) + O(lch1).
            # pair1's O accumulations lag (3 and 5 slots) so the pair0 norms
            # have PE cover before pair1's matmuls alias pair0's PSUM banks.
            o_ps0 = {}   # pair -> [o_ps hh0, hh1] for lch0
            o_ps1 = {}
            with tc.tile_pool(name="ps_o", bufs=4, space="PSUM") as pso:
                def o_tiles(lch, pair):
                    return [pso.tile([D + 1, LCH], f32, tag="oo",
                                     name=f"ops{lch}_{pair}_{h}")
                            for h in range(2)]

                o_ps0[0] = o_tiles(0, 0)
                o_ps1[0] = o_tiles(1, 0)
                for s in range(NST):
                    # O steps first: the ST write below reuses the pt buffer
                    # freed by an earlier slot's O reads (pool rotation).
                    o_step(o_ps0[0], s, 0, pt0.pop((0, s)))
                    if s >= 2:
                        o_step(o_ps1[0], s - 2, 0, pt1.pop((0, s - 2)))
                    pt1[(0, s)] = st_step(1, 0, s)
                norm_lch(0, 0, o_ps0[0])
                o_ps0[1] = o_tiles(0, 1)
                o_ps1[1] = o_tiles(1, 1)
                for s in range(NST):
                    if s < 2:
                        o_step(o_ps1[0], 14 + s, 0, pt1.pop((0, 14 + s)))
                    if s == 1:
                        norm_lch(0, 1, o_ps1[0])
                    if s >= 3:
                        o_step(o_ps0[1], s - 3, 1, pt0.pop((1, s - 3)))
                    if s >= 5:
                        o_step(o_ps1[1], s - 5, 1, pt1.pop((1, s - 5)))
                    pt1[(1, s)] = st_step(1, 1, s)
                for s in range(13, 16):
                    o_step(o_ps0[1], s, 1, pt0.pop((1, s)))
                norm_lch(1, 0, o_ps0[1], tail=True)
                for s in range(11, 16):
                    o_step(o_ps1[1], s, 1, pt1.pop((1, s)))
                norm_lch(1, 1, o_ps1[1], tail=True)

            pst_cm.__exit__(None, None, None)

            # =========== Wo tail: lch0 jobs first (unblocked earlier) =======
            with tc.tile_pool(name="ps_wo", bufs=6, space="PSUM") as pswo:
                ncast = 0
                for lt in list(range(4)) + list(range(4, 8)):
                    for nch in range(2):
                        wo_step(pswo, lt, nch,
                                "scalar" if ncast % 2 == 0 else "vector")
                        ncast += 1

    nc.compile()
    return nc


def _get_nc():
    if "nc" not in _cache:
        _cache["nc"] = _build()
    return _cache["nc"]


def _make_in_maps(inputs):
    import ml_dtypes

    bf16 = ml_dtypes.bfloat16
    query = np.asarray(inputs["query"], dtype=np.float32)
    key = np.asarray(inputs["key"], dtype=np.float32)
    value = np.asarray(inputs["value"], dtype=np.float32)
    Wq = np.asarray(inputs["Wq"], dtype=np.float32)
    Wk = np.asarray(inputs["Wk"], dtype=np.float32)
    Wv = np.asarray(inputs["Wv"], dtype=np.float32)
    Wo = np.asarray(inputs["Wo"], dtype=np.float32)

    qT = [np.ascontiguousarray(query[b].T).astype(bf16) for b in range(B)]
    kT = [np.ascontiguousarray(key[b].T).astype(bf16) for b in range(B)]
    vT = [np.ascontiguousarray(value[b].T).astype(bf16) for b in range(B)]
    wq_s = [np.ascontiguousarray(Wq[:, g * M:(g + 1) * M]).astype(bf16) for g in range(4)]
    wk_s = [np.ascontiguousarray(Wk[:, g * M:(g + 1) * M]).astype(bf16) for g in range(4)]
    wv_s = [np.ascontiguousarray(Wv[:, g * M:(g + 1) * M]).astype(bf16) for g in range(4)]
    wo_s = [np.ascontiguousarray(Wo[g * M:(g + 1) * M, :]).astype(bf16) for g in range(4)]

    in_maps = []
    for core in range(NCORES):
        b, g = core // 4, core % 4
        in_maps.append({
            "xqT": qT[b], "xkT": kT[b], "xvT": vT[b],
            "wq": wq_s[g], "wk": wk_s[g], "wv": wv_s[g], "wo": wo_s[g],
        })
    return in_maps


def kernel(query, key, value, Wq, Wk, Wv, Wo, bo):
    from concourse.bass_utils import run_bass_kernel_spmd

    nc = _get_nc()
    bo = np.asarray(bo, dtype=np.float32)
    in_maps = _make_in_maps(dict(query=query, key=key, value=value,
                                 Wq=Wq, Wk=Wk, Wv=Wv, Wo=Wo))

    res = run_bass_kernel_spmd(nc, in_maps, core_ids=list(range(NCORES)))

    out = np.zeros((B, L, C), dtype=np.float32)
    for core in range(NCORES):
        b = core // 4
        out[b] += np.asarray(res.results[core]["outp"], dtype=np.float32)
    out += bo[None, None, :]
    return out


# revision 13
# speedup vs baseline: 1.0274x; 1.0270x over previous
"""Trainium2 Bass kernel for nn_CrossAttention (B=2, L=1024, S=2048, DIM=1024, H=16 heads).

Sharding: tensor-parallel over heads x data-parallel over batch.
Core c handles batch b = c//4 and head-group g = c%4 (4 heads = 256 of the
1024 hidden channels).  Each core computes, for its (b, g):

    QT = (Wq_g)^T x_q^T          [256, 1024]   (m on partitions)
    KT = (Wk_g)^T x_k^T          [256, 2048]
    V  = x_v Wv_g                [2048, 256]   (s on partitions)
    per head h (d=64):
        ST_h = KT_h^T' ...       S^T[s, l] = k_s . q_l   (s on partitions)
        P_h  = exp(SCALE * ST_h)            (unnormalized, s on partitions)
        [O^T_h ; sums_h] = [V_h | 1]^T @ P_h   (ones-column folds the softmax
                                                denominator into the matmul)
        XgT_h = O^T_h * (1/sums_h)          (gpsimd partition broadcast)
    out_partial = XgT^T @ Wo_g   [1024, 1024]

Host gathers: out[b] = sum_g out_partial[4b+g] + bo.

v3 schedule: the kernel is jointly PE- and ACT(exp)-bound; the structure
minimizes time-to-first-exp, keeps the 64-exp stream dense, and minimizes
the serial tail after the last exp:
  - DMAs only move from ~8.7us (runtime preamble), so the pre-exp bytes are
    cut to 2.5MB: wq, xq l-half 0, wk, first 256-wide xk slice.  xk arrives
    as 5 s-slices so ST steps unlock progressively; xq l-half 1 is deferred
    behind the first three xk slices (lch1 STs don't run until the merged
    phase).  qt/kt/xgt live as per-region tiles so readers never see
    false whole-tile dependencies on later writers.
  - ~20 warmup matmuls keep the PE HAM clock-gate busy (2.4GHz) until real
    data lands; phase A then runs DMA-paced with pair-0 ST steps fired as
    each kt slice is cast.  Phase B (V projection) paces the pair-1 steps.
  - Merged phase: per slot: O(lch0) + O(lch1, lagged) + ST(lch1); pair1's
    O accumulations lag 3/5 slots so the pair0 norms get PE cover before
    pair1's matmuls alias pair0's PSUM banks.  Both O accumulations finish
    with the exp stream, so the post-exp tail is only norm + Wo.
  - Wo PSUM opens after the ST pool closes (its banks are gated only by the
    last exp reads, not the norms); jobs run lch0-first, casts alternate
    scalar/vector engines.
Output partials are written bf16 (host accumulates fp32); the softmax
reciprocal uses the fast approximate DVE op + gpsimd partition broadcast.
"""

import sys

if "/opt/trn_rl_repo" not in sys.path:
    sys.path.insert(0, "/opt/trn_rl_repo")

import numpy as np

B, L, S, C = 2, 1024, 2048, 1024
NH, D = 16, 64          # total heads, head dim
HPC = 4                 # heads per core
M = HPC * D             # 256 output channels per core
SCALE = D ** -0.5
P = 128                 # partitions
NCORES = 8
CK = C // P             # 8 c-tiles
NST = S // P            # 16 s-tiles
LCH = 512               # l-chunk
NLCH = L // LCH         # 2

# xk s-slice widths (elements of s); first two narrow so the first ST
# steps unlock with minimal DMA.
KSL = [256, 256, 512, 512, 512]
KSL_OFF = [0, 256, 512, 1024, 1536]
# st tile -> (slice index, offset within slice)
ST_SLICE = []
for _i, (_o, _w) in enumerate(zip(KSL_OFF, KSL)):
    for _j in range(_w // P):
        ST_SLICE.append((_i, _j * P))

_cache = {}


def _build():
    import concourse.tile as tile
    from concourse import mybir, bacc

    f32 = mybir.dt.float32
    bf16 = mybir.dt.bfloat16

    nc = bacc.Bacc("TRN2", target_bir_lowering=False, debug=False)

    xqT = nc.dram_tensor("xqT", [C, L], bf16, kind="ExternalInput")
    xkT = nc.dram_tensor("xkT", [C, S], bf16, kind="ExternalInput")
    xvT = nc.dram_tensor("xvT", [C, S], bf16, kind="ExternalInput")
    wq = nc.dram_tensor("wq", [C, M], bf16, kind="ExternalInput")
    wk = nc.dram_tensor("wk", [C, M], bf16, kind="ExternalInput")
    wv = nc.dram_tensor("wv", [C, M], bf16, kind="ExternalInput")
    wo = nc.dram_tensor("wo", [M, C], bf16, kind="ExternalInput")
    outp = nc.dram_tensor("outp", [L, C], bf16, kind="ExternalOutput")

    with tile.TileContext(nc) as tc:
        with tc.tile_pool(name="singles", bufs=1) as singles, \
             tc.tile_pool(name="xk_pool", bufs=3) as xkp, \
             tc.tile_pool(name="xv_pool", bufs=3) as xvp, \
             tc.tile_pool(name="pts", bufs=36) as pts, \
             tc.tile_pool(name="small", bufs=3) as small, \
             tc.tile_pool(name="obuf", bufs=4) as obuf:

            # ---- persistent SBUF ----
            wq_sb = singles.tile([P, CK, M], bf16, tag="wq")
            wk_sb = singles.tile([P, CK, M], bf16, tag="wk")
            wv_sb = singles.tile([P, CK, M], bf16, tag="wv")
            wo_sb = singles.tile([P, M // P, C], bf16, tag="wo")
            xq_sb = singles.tile([P, CK, L], bf16, tag="xq")

            # DMA priority order = consumption order.
            nc.sync.dma_start(wq_sb[:], wq.rearrange("(ck p) m -> p ck m", p=P))
            nc.sync.dma_start(xq_sb[:, :, 0:LCH],
                              xqT[:, 0:LCH].rearrange("(ck p) l -> p ck l", p=P))
            nc.sync.dma_start(wk_sb[:], wk.rearrange("(ck p) m -> p ck m", p=P))
            xk_t = []

            def load_xk(i):
                t = xkp.tile([P, CK, KSL[i]], bf16, tag="xk", name=f"xk{i}")
                nc.sync.dma_start(
                    t[:], xkT[:, KSL_OFF[i]:KSL_OFF[i] + KSL[i]]
                    .rearrange("(ck p) s -> p ck s", p=P))
                xk_t.append(t)

            load_xk(0)
            load_xk(1)
            load_xk(2)
            nc.sync.dma_start(xq_sb[:, :, LCH:L],
                              xqT[:, LCH:L].rearrange("(ck p) l -> p ck l", p=P))
            load_xk(3)
            load_xk(4)
            nc.sync.dma_start(wv_sb[:], wv.rearrange("(ck p) m -> p ck m", p=P))
            xv_t = []
            for q in range(4):
                t = xvp.tile([P, CK, 4 * P], bf16, tag="xv", name=f"xv{q}")
                nc.sync.dma_start(
                    t[:], xvT[:, q * 4 * P:(q + 1) * 4 * P]
                    .rearrange("(ck p) s -> p ck s", p=P))
                xv_t.append(t)
            nc.sync.dma_start(wo_sb[:], wo.rearrange("(kt p) n -> p kt n", p=P))

            # qt/kt/xgt as per-region tiles (no false whole-tile deps)
            qt_t = [singles.tile([P, 2, LCH], bf16, tag=f"qt{lh}",
                                 name=f"qt{lh}")
                    for lh in range(2)]                    # [m%128, m//128, l]
            kt_t = [singles.tile([P, 2, KSL[i]], bf16, tag=f"kt{i}",
                                 name=f"kt{i}")
                    for i in range(5)]                     # [m%128, m//128, s]
            vones = singles.tile([P, NST, HPC, D + 1], bf16, tag="vones")
            xgt_t = [singles.tile([P, 2, LCH], bf16, tag=f"xgt{lc}",
                                  name=f"xgt{lc}")
                     for lc in range(2)]
            stage = singles.tile([P, D], f32, tag="stage")
            nc.vector.memset(stage[:], 1.0)
            nc.vector.tensor_copy(vones[:, :, :, D:D + 1],
                                  stage[:].rearrange("p (a b) -> p a b", a=NST)[:, :, :, None])

            # ---- step helpers ----
            def st_step(lch, pair, st):
                """ST pair matmuls + exp; returns the PT tile."""
                sl, so = ST_SLICE[st]
                st_ps = pst.tile([P, 2, LCH], f32, tag="st", name=f"stps_{lch}_{pair}_{st}")
                nc.tensor.matmul(
                    st_ps[:, 0, :], kt_t[sl][0:D, pair, so:so + P],
                    qt_t[lch][0:D, pair, :], start=True, stop=True)
                nc.tensor.matmul(
                    st_ps[:, 1, :], kt_t[sl][D:P, pair, so:so + P],
                    qt_t[lch][D:P, pair, :], start=True, stop=True,
                    tile_position=(64, 0))
                pt_t = pts.tile([P, 2, LCH], bf16, tag="pt", name=f"pt_{lch}_{pair}_{st}")
                nc.scalar.activation(pt_t[:], st_ps[:],
                                     mybir.ActivationFunctionType.Exp, scale=SCALE)
                return pt_t

            def o_step(o_ps, st, pair, pt_t):
                for hh in range(2):
                    nc.tensor.matmul(
                        o_ps[hh][:], vones[:, st, pair * 2 + hh, :], pt_t[:, hh, :],
                        start=(st == 0), stop=(st == NST - 1))

            def norm_lch(pair, lch, o_ps, tail=False):
                """per-(pair,lch) softmax normalize: fast reciprocal of the
                sums row -> gpsimd partition broadcast -> scaled XgT."""
                for hh in range(2):
                    sums_sb = small.tile([1, LCH], f32, tag="sums")
                    if tail and hh == 1:
                        nc.scalar.copy(sums_sb[:], o_ps[hh][D:D + 1, :])
                    else:
                        nc.vector.tensor_copy(sums_sb[:], o_ps[hh][D:D + 1, :])
                    rc = small.tile([1, LCH], f32, tag="rc")
                    nc.vector.reciprocal_approx_fast(rc[:], sums_sb[:])
                    bc_sb = small.tile([D, LCH], f32, tag="bc")
                    nc.gpsimd.partition_broadcast(bc_sb[:], rc[:])
                    nc.vector.tensor_mul(
                        xgt_t[lch][hh * D:(hh + 1) * D, pair, :],
                        o_ps[hh][0:D, :], bc_sb[:])

            def wo_step(pool, lt, nch, cast_eng):
                wo_ps = pool.tile([P, 512], f32, tag="wo", name=f"wops_{lt}_{nch}")
                for kt in range(2):
                    nc.tensor.matmul(
                        wo_ps[:], xgt_t[lt // 4][:, kt, (lt % 4) * P:(lt % 4 + 1) * P],
                        wo_sb[:, kt, nch * 512:(nch + 1) * 512],
                        start=(kt == 0), stop=(kt == 1))
                ob_sb = obuf.tile([P, 512], bf16, tag="ob")
                if cast_eng == "scalar":
                    nc.scalar.copy(ob_sb[:], wo_ps[:])
                    nc.scalar.dma_start(
                        outp[lt * P:(lt + 1) * P, nch * 512:(nch + 1) * 512], ob_sb[:])
                else:
                    nc.vector.tensor_copy(ob_sb[:], wo_ps[:])
                    nc.gpsimd.dma_start(
                        outp[lt * P:(lt + 1) * P, nch * 512:(nch + 1) * 512], ob_sb[:])

            # ---- PSUM pool timeline:
            #   pst(4) > [psw(1) warmup] > [psp(4) A] > [psv(4) B] >
            #   [pso(4) merged]; pst closes after the last ST, then ps_wo(4)
            #   reuses pst's banks (gated only by the last exp reads).
            pst_cm = tc.tile_pool(name="ps_st", bufs=2, space="PSUM")
            pst = pst_cm.__enter__()

            pt0 = {}   # (pair, st) -> PT tile for lch 0
            pt1 = {}

            # PE warm-up: keep the HAM clock-gate busy until real data lands.
            warm_sb = singles.tile([P, 512], bf16, tag="warm")
            nc.vector.memset(warm_sb[:], 1.0)
            with tc.tile_pool(name="ps_warm", bufs=1, space="PSUM") as psw:
                warm = psw.tile([P, 512], f32, tag="warm")
                for i in range(20):
                    nc.tensor.matmul(
                        warm[:], warm_sb[:, 0:128], warm_sb[:],
                        start=True, stop=True)

            # =========== Phase A: projections + pair-0 ST steps =============
            with tc.tile_pool(name="ps_proj", bufs=4, space="PSUM") as psp:

                def q_pass(lh):
                    q_ps = [psp.tile([P, 512], f32, tag="pp", name=f"qtps{lh}_{mt}")
                            for mt in range(2)]
                    for ck in range(CK):
                        for mt in range(2):
                            nc.tensor.matmul(
                                q_ps[mt][:],
                                wq_sb[:, ck, mt * P:(mt + 1) * P],
                                xq_sb[:, ck, lh * 512:(lh + 1) * 512],
                                start=(ck == 0), stop=(ck == CK - 1))
                    for mt in range(2):
                        nc.vector.tensor_copy(qt_t[lh][:, mt, :], q_ps[mt][:])

                def k_pass(sl):
                    w = KSL[sl]
                    k_ps = [psp.tile([P, w], f32, tag="pp", name=f"ktps{sl}_{mt}")
                            for mt in range(2)]
                    for ck in range(CK):
                        for mt in range(2):
                            nc.tensor.matmul(
                                k_ps[mt][:],
                                wk_sb[:, ck, mt * P:(mt + 1) * P],
                                xk_t[sl][:, ck, :],
                                start=(ck == 0), stop=(ck == CK - 1))
                    for mt in range(2):
                        nc.vector.tensor_copy(kt_t[sl][:, mt, :], k_ps[mt][:])

                def fire(sl):
                    first = KSL_OFF[sl] // P
                    for st in range(first, first + KSL[sl] // P):
                        pt0[(0, st)] = st_step(0, 0, st)

                q_pass(0)
                k_pass(0)
                fire(0)          # st 0-1
                k_pass(1)
                fire(1)          # st 2-3
                k_pass(2)
                fire(2)          # st 4-7
                q_pass(1)
                k_pass(3)
                fire(3)          # st 8-11
                k_pass(4)
                fire(4)          # st 12-15

            # =========== Phase B: V projection + pair-1 lch0 ST steps =======
            b_jobs = [(1, st) for st in range(16)]
            with tc.tile_pool(name="ps_v", bufs=4, space="PSUM") as psv:
                for q in range(4):
                    for half in range(2):
                        v_ps = [psv.tile([P, M], f32, tag="vv",
                                         name=f"vps{q}_{half}_{i}")
                                for i in range(2)]
                        for ck in range(CK):
                            for st2 in range(2):
                                st4 = half * 2 + st2
                                nc.tensor.matmul(
                                    v_ps[st2][:],
                                    xv_t[q][:, ck, st4 * P:(st4 + 1) * P],
                                    wv_sb[:, ck, :],
                                    start=(ck == 0), stop=(ck == CK - 1))
                            if ck in (3, 7) and b_jobs:
                                pair, st = b_jobs.pop(0)
                                pt0[(pair, st)] = st_step(0, pair, st)
                        for st2 in range(2):
                            st = q * 4 + half * 2 + st2
                            nc.vector.tensor_copy(
                                vones[:, st, :, 0:D],
                                v_ps[st2][:].rearrange("p (h d) -> p h d", h=HPC))
                for pair, st in b_jobs:
                    pt0[(pair, st)] = st_step(0, pair, st)

            # ====== Merged phase: per slot: O(lch0) + O(lch1) + ST(lch1).
            # pair1's O accumulations lag (3 and 5 slots) so the pair0 norms
            # have PE cover before pair1's matmuls alias pair0's PSUM banks.
            o_ps0 = {}   # pair -> [o_ps hh0, hh1] for lch0
            o_ps1 = {}
            # right side: independent pool stack, so pst (left) can close
            # before the norms finish and ps_wo reuses pst's banks.
            with tc.tile_pool(name="ps_o", bufs=4, space="PSUM",
                              side="right") as pso:
                def o_tiles(lch, pair):
                    return [pso.tile([D + 1, LCH], f32, tag="oo",
                                     name=f"ops{lch}_{pair}_{h}")
                            for h in range(2)]

                o_ps0[0] = o_tiles(0, 0)
                o_ps1[0] = o_tiles(1, 0)
                for s in range(NST):
                    # O steps first: the ST write below reuses the pt buffer
                    # freed by an earlier slot's O reads (pool rotation).
                    o_step(o_ps0[0], s, 0, pt0.pop((0, s)))
                    if s >= 2:
                        o_step(o_ps1[0], s - 2, 0, pt1.pop((0, s - 2)))
                    pt1[(0, s)] = st_step(1, 0, s)
                norm_lch(0, 0, o_ps0[0])
                o_ps0[1] = o_tiles(0, 1)
                o_ps1[1] = o_tiles(1, 1)
                for s in range(NST):
                    if s < 2:
                        o_step(o_ps1[0], 14 + s, 0, pt1.pop((0, 14 + s)))
                    if s == 1:
                        norm_lch(0, 1, o_ps1[0])
                    if s >= 3:
                        o_step(o_ps0[1], s - 3, 1, pt0.pop((1, s - 3)))
                    if s >= 5:
                        o_step(o_ps1[1], s - 5, 1, pt1.pop((1, s - 5)))
                    pt1[(1, s)] = st_step(1, 1, s)
                # drains + tail norms (Wo-lch0 unblocks after norm(1,0))
                for s in range(13, 16):
                    o_step(o_ps0[1], s, 1, pt0.pop((1, s)))
                norm_lch(1, 0, o_ps0[1], tail=True)
                for s in range(11, 16):
                    o_step(o_ps1[1], s, 1, pt1.pop((1, s)))
                norm_lch(1, 1, o_ps1[1], tail=True)

                pst_cm.__exit__(None, None, None)

                # ======= Wo tail: lch0 jobs first (unblocked earlier) =======
                with tc.tile_pool(name="ps_wo", bufs=4, space="PSUM") as pswo:
                    ncast = 0
                    for lt in range(8):
                        for nch in range(2):
                            wo_step(pswo, lt, nch,
                                    "scalar" if ncast % 2 == 0 else "vector")
                            ncast += 1

    nc.compile()
    return nc


def _get_nc():
    if "nc" not in _cache:
        _cache["nc"] = _build()
    return _cache["nc"]


def _make_in_maps(inputs):
    import ml_dtypes

    bf16 = ml_dtypes.bfloat16
    query = np.asarray(inputs["query"], dtype=np.float32)
    key = np.asarray(inputs["key"], dtype=np.float32)
    value = np.asarray(inputs["value"], dtype=np.float32)
    Wq = np.asarray(inputs["Wq"], dtype=np.float32)
    Wk = np.asarray(inputs["Wk"], dtype=np.float32)
    Wv = np.asarray(inputs["Wv"], dtype=np.float32)
    Wo = np.asarray(inputs["Wo"], dtype=np.float32)

    qT = [np.ascontiguousarray(query[b].T).astype(bf16) for b in range(B)]
    kT = [np.ascontiguousarray(key[b].T).astype(bf16) for b in range(B)]
    vT = [np.ascontiguousarray(value[b].T).astype(bf16) for b in range(B)]
    wq_s = [np.ascontiguousarray(Wq[:, g * M:(g + 1) * M]).astype(bf16) for g in range(4)]
    wk_s = [np.ascontiguousarray(Wk[:, g * M:(g + 1) * M]).astype(bf16) for g in range(4)]
    wv_s = [np.ascontiguousarray(Wv[:, g * M:(g + 1) * M]).astype(bf16) for g in range(4)]
    wo_s = [np.ascontiguousarray(Wo[g * M:(g + 1) * M, :]).astype(bf16) for g in range(4)]

    in_maps = []
    for core in range(NCORES):
        b, g = core // 4, core % 4
        in_maps.append({
            "xqT": qT[b], "xkT": kT[b], "xvT": vT[b],
            "wq": wq_s[g], "wk": wk_s[g], "wv": wv_s[g], "wo": wo_s[g],
        })
    return in_maps


def kernel(query, key, value, Wq, Wk, Wv, Wo, bo):
    from concourse.bass_utils import run_bass_kernel_spmd

    nc = _get_nc()
    bo = np.asarray(bo, dtype=np.float32)
    in_maps = _make_in_maps(dict(query=query, key=key, value=value,
                                 Wq=Wq, Wk=Wk, Wv=Wv, Wo=Wo))

    res = run_bass_kernel_spmd(nc, in_maps, core_ids=list(range(NCORES)))

    out = np.zeros((B, L, C), dtype=np.float32)
    for core in range(NCORES):
        b = core // 4
        out[b] += np.asarray(res.results[core]["outp"], dtype=np.float32)
    out += bo[None, None, :]
    return out


# revision 19
# speedup vs baseline: 1.0294x; 1.0019x over previous
"""Trainium2 Bass kernel for nn_CrossAttention (B=2, L=1024, S=2048, DIM=1024, H=16 heads).

Sharding: tensor-parallel over heads x data-parallel over batch.
Core c handles batch b = c//4 and head-group g = c%4 (4 heads = 256 of the
1024 hidden channels).  Each core computes, for its (b, g):

    QT = (Wq_g)^T x_q^T          [256, 1024]   (m on partitions)
    KT = (Wk_g)^T x_k^T          [256, 2048]
    V  = x_v Wv_g                [2048, 256]   (s on partitions)
    per head h (d=64):
        ST_h = KT_h^T' ...       S^T[s, l] = k_s . q_l   (s on partitions)
        P_h  = exp(SCALE * ST_h)            (unnormalized, s on partitions)
        [O^T_h ; sums_h] = [V_h | 1]^T @ P_h   (ones-column folds the softmax
                                                denominator into the matmul)
        XgT_h = O^T_h * (1/sums_h)          (gpsimd partition broadcast)
    out_partial = XgT^T @ Wo_g   [1024, 1024]

Host gathers: out[b] = sum_g out_partial[4b+g] + bo.

v3 schedule: the kernel is jointly PE- and ACT(exp)-bound; the structure
minimizes time-to-first-exp, keeps the 64-exp stream dense, and minimizes
the serial tail after the last exp:
  - DMAs only move from ~8.7us (runtime preamble), so the pre-exp bytes are
    cut to 2.5MB: wq, xq l-half 0, wk, first 256-wide xk slice.  xk arrives
    as 5 s-slices so ST steps unlock progressively; xq l-half 1 is deferred
    behind the first three xk slices (lch1 STs don't run until the merged
    phase).  qt/kt/xgt live as per-region tiles so readers never see
    false whole-tile dependencies on later writers.
  - ~20 warmup matmuls keep the PE HAM clock-gate busy (2.4GHz) until real
    data lands; phase A then runs DMA-paced with pair-0 ST steps fired as
    each kt slice is cast.  Phase B (V projection) paces the pair-1 steps.
  - Merged phase: per slot: O(lch0) + O(lch1, lagged) + ST(lch1); pair1's
    O accumulations lag 3/5 slots so the pair0 norms get PE cover before
    pair1's matmuls alias pair0's PSUM banks.  Both O accumulations finish
    with the exp stream, so the post-exp tail is only norm + Wo.
  - Wo PSUM opens after the ST pool closes (its banks are gated only by the
    last exp reads, not the norms); jobs run lch0-first, casts alternate
    scalar/vector engines.
Output partials are written bf16 (host accumulates fp32); the softmax
reciprocal uses the fast approximate DVE op + gpsimd partition broadcast.
"""

import sys

if "/opt/trn_rl_repo" not in sys.path:
    sys.path.insert(0, "/opt/trn_rl_repo")

import numpy as np

B, L, S, C = 2, 1024, 2048, 1024
NH, D = 16, 64          # total heads, head dim
HPC = 4                 # heads per core
M = HPC * D             # 256 output channels per core
SCALE = D ** -0.5
P = 128                 # partitions
NCORES = 8
CK = C // P             # 8 c-tiles
NST = S // P            # 16 s-tiles
LCH = 512               # l-chunk
NLCH = L // LCH         # 2

# xk s-slice widths (elements of s); first two narrow so the first ST
# steps unlock with minimal DMA.
KSL = [256, 256, 512, 512, 512]
KSL_OFF = [0, 256, 512, 1024, 1536]
# st tile -> (slice index, offset within slice)
ST_SLICE = []
for _i, (_o, _w) in enumerate(zip(KSL_OFF, KSL)):
    for _j in range(_w // P):
        ST_SLICE.append((_i, _j * P))

_cache = {}


def _build():
    import concourse.tile as tile
    from concourse import mybir, bacc

    f32 = mybir.dt.float32
    bf16 = mybir.dt.bfloat16

    nc = bacc.Bacc("TRN2", target_bir_lowering=False, debug=False)

    xqT = nc.dram_tensor("xqT", [C, L], bf16, kind="ExternalInput")
    xkT = nc.dram_tensor("xkT", [C, S], bf16, kind="ExternalInput")
    xvT = nc.dram_tensor("xvT", [C, S], bf16, kind="ExternalInput")
    wq = nc.dram_tensor("wq", [C, M], bf16, kind="ExternalInput")
    wk = nc.dram_tensor("wk", [C, M], bf16, kind="ExternalInput")
    wv = nc.dram_tensor("wv", [C, M], bf16, kind="ExternalInput")
    wo = nc.dram_tensor("wo", [M, C], bf16, kind="ExternalInput")
    outp = nc.dram_tensor("outp", [L, C], bf16, kind="ExternalOutput")

    with tile.TileContext(nc) as tc:
        with tc.tile_pool(name="singles", bufs=1) as singles, \
             tc.tile_pool(name="xk_pool", bufs=3) as xkp, \
             tc.tile_pool(name="xv_pool", bufs=3) as xvp, \
             tc.tile_pool(name="pts", bufs=36) as pts, \
             tc.tile_pool(name="small", bufs=3) as small, \
             tc.tile_pool(name="obuf", bufs=4) as obuf:

            # ---- persistent SBUF ----
            wq_sb = singles.tile([P, CK, M], bf16, tag="wq")
            wk_sb = singles.tile([P, CK, M], bf16, tag="wk")
            wv_sb = singles.tile([P, CK, M], bf16, tag="wv")
            wo_sb = singles.tile([P, M // P, C], bf16, tag="wo")
            xq_sb = singles.tile([P, CK, L], bf16, tag="xq")

            # DMA priority order = consumption order: the first ST+exp needs
            # only wk + xk slice 0 + wq + xq l-half 0 (2.5MB).
            xk_t = []

            def load_xk(i):
                t = xkp.tile([P, CK, KSL[i]], bf16, tag="xk", name=f"xk{i}")
                nc.sync.dma_start(
                    t[:], xkT[:, KSL_OFF[i]:KSL_OFF[i] + KSL[i]]
                    .rearrange("(ck p) s -> p ck s", p=P))
                xk_t.append(t)

            nc.sync.dma_start(wk_sb[:], wk.rearrange("(ck p) m -> p ck m", p=P))
            load_xk(0)
            nc.sync.dma_start(wq_sb[:], wq.rearrange("(ck p) m -> p ck m", p=P))
            nc.sync.dma_start(xq_sb[:, :, 0:LCH],
                              xqT[:, 0:LCH].rearrange("(ck p) l -> p ck l", p=P))
            load_xk(1)
            load_xk(2)
            load_xk(3)
            load_xk(4)
            nc.sync.dma_start(xq_sb[:, :, LCH:L],
                              xqT[:, LCH:L].rearrange("(ck p) l -> p ck l", p=P))
            nc.sync.dma_start(wv_sb[:], wv.rearrange("(ck p) m -> p ck m", p=P))
            xv_t = []
            for q in range(4):
                t = xvp.tile([P, CK, 4 * P], bf16, tag="xv", name=f"xv{q}")
                nc.sync.dma_start(
                    t[:], xvT[:, q * 4 * P:(q + 1) * 4 * P]
                    .rearrange("(ck p) s -> p ck s", p=P))
                xv_t.append(t)
            nc.sync.dma_start(wo_sb[:], wo.rearrange("(kt p) n -> p kt n", p=P))

            # qt/kt/xgt as per-region tiles (no false whole-tile deps)
            qt_t = [singles.tile([P, 2, LCH], bf16, tag=f"qt{lh}",
                                 name=f"qt{lh}")
                    for lh in range(2)]                    # [m%128, m//128, l]
            kt_t = [singles.tile([P, 2, KSL[i]], bf16, tag=f"kt{i}",
                                 name=f"kt{i}")
                    for i in range(5)]                     # [m%128, m//128, s]
            vones = singles.tile([P, NST, HPC, D + 1], bf16, tag="vones")
            xgt_t = [singles.tile([P, 2, LCH], bf16, tag=f"xgt{lc}",
                                  name=f"xgt{lc}")
                     for lc in range(2)]
            stage = singles.tile([P, D], f32, tag="stage")
            nc.vector.memset(stage[:], 1.0)
            nc.vector.tensor_copy(vones[:, :, :, D:D + 1],
                                  stage[:].rearrange("p (a b) -> p a b", a=NST)[:, :, :, None])

            # ---- step helpers ----
            def st_step(lch, pair, st):
                """ST pair matmuls + exp; returns the PT tile."""
                sl, so = ST_SLICE[st]
                st_ps = pst.tile([P, 2, LCH], f32, tag="st", name=f"stps_{lch}_{pair}_{st}")
                nc.tensor.matmul(
                    st_ps[:, 0, :], kt_t[sl][0:D, pair, so:so + P],
                    qt_t[lch][0:D, pair, :], start=True, stop=True)
                nc.tensor.matmul(
                    st_ps[:, 1, :], kt_t[sl][D:P, pair, so:so + P],
                    qt_t[lch][D:P, pair, :], start=True, stop=True,
                    tile_position=(64, 0))
                pt_t = pts.tile([P, 2, LCH], bf16, tag="pt", name=f"pt_{lch}_{pair}_{st}")
                nc.scalar.activation(pt_t[:], st_ps[:],
                                     mybir.ActivationFunctionType.Exp, scale=SCALE)
                return pt_t

            def o_step(o_ps, st, pair, pt_t):
                for hh in range(2):
                    nc.tensor.matmul(
                        o_ps[hh][:], vones[:, st, pair * 2 + hh, :], pt_t[:, hh, :],
                        start=(st == 0), stop=(st == NST - 1))

            def norm_lch(pair, lch, o_ps, tail=False):
                """per-(pair,lch) softmax normalize: fast reciprocal of the
                sums row -> gpsimd partition broadcast -> scaled XgT."""
                for hh in range(2):
                    sums_sb = small.tile([1, LCH], f32, tag="sums")
                    if tail:
                        # scalar engine is idle post-exp; keep the DVE free
                        # for the reciprocals/multiplies on the critical tail
                        nc.scalar.copy(sums_sb[:], o_ps[hh][D:D + 1, :])
                    else:
                        nc.vector.tensor_copy(sums_sb[:], o_ps[hh][D:D + 1, :])
                    rc = small.tile([1, LCH], f32, tag="rc")
                    nc.vector.reciprocal_approx_fast(rc[:], sums_sb[:])
                    bc_sb = small.tile([D, LCH], f32, tag="bc")
                    nc.gpsimd.partition_broadcast(bc_sb[:], rc[:])
                    nc.vector.tensor_mul(
                        xgt_t[lch][hh * D:(hh + 1) * D, pair, :],
                        o_ps[hh][0:D, :], bc_sb[:])

            def wo_step(pool, lt, nch, cast_eng):
                wo_ps = pool.tile([P, 512], f32, tag="wo", name=f"wops_{lt}_{nch}")
                for kt in range(2):
                    nc.tensor.matmul(
                        wo_ps[:], xgt_t[lt // 4][:, kt, (lt % 4) * P:(lt % 4 + 1) * P],
                        wo_sb[:, kt, nch * 512:(nch + 1) * 512],
                        start=(kt == 0), stop=(kt == 1))
                ob_sb = obuf.tile([P, 512], bf16, tag="ob")
                if cast_eng == "scalar":
                    nc.scalar.copy(ob_sb[:], wo_ps[:])
                    nc.scalar.dma_start(
                        outp[lt * P:(lt + 1) * P, nch * 512:(nch + 1) * 512], ob_sb[:])
                else:
                    nc.vector.tensor_copy(ob_sb[:], wo_ps[:])
                    nc.gpsimd.dma_start(
                        outp[lt * P:(lt + 1) * P, nch * 512:(nch + 1) * 512], ob_sb[:])

            # ---- PSUM pool timeline:
            #   pst(4) > [psw(1) warmup] > [psp(4) A] > [psv(4) B] >
            #   [pso(4) merged]; pst closes after the last ST, then ps_wo(4)
            #   reuses pst's banks (gated only by the last exp reads).
            pst_cm = tc.tile_pool(name="ps_st", bufs=2, space="PSUM")
            pst = pst_cm.__enter__()

            pt0 = {}   # (pair, st) -> PT tile for lch 0
            pt1 = {}

            # PE warm-up: keep the HAM clock-gate busy until real data lands.
            warm_sb = singles.tile([P, 512], bf16, tag="warm")
            nc.vector.memset(warm_sb[:], 1.0)
            with tc.tile_pool(name="ps_warm", bufs=1, space="PSUM") as psw:
                warm = psw.tile([P, 512], f32, tag="warm")
                for i in range(8):
                    nc.tensor.matmul(
                        warm[:], warm_sb[:, 0:128], warm_sb[:],
                        start=True, stop=True)

            # =========== Phase A: projections + pair-0 ST steps =============
            with tc.tile_pool(name="ps_proj", bufs=4, space="PSUM") as psp:

                def q_pass(lh):
                    q_ps = [psp.tile([P, 512], f32, tag="pp", name=f"qtps{lh}_{mt}")
                            for mt in range(2)]
                    for ck in range(CK):
                        for mt in range(2):
                            nc.tensor.matmul(
                                q_ps[mt][:],
                                wq_sb[:, ck, mt * P:(mt + 1) * P],
                                xq_sb[:, ck, lh * 512:(lh + 1) * 512],
                                start=(ck == 0), stop=(ck == CK - 1))
                    for mt in range(2):
                        nc.vector.tensor_copy(qt_t[lh][:, mt, :], q_ps[mt][:])

                def k_pass(sl):
                    w = KSL[sl]
                    k_ps = [psp.tile([P, w], f32, tag="pp", name=f"ktps{sl}_{mt}")
                            for mt in range(2)]
                    for ck in range(CK):
                        for mt in range(2):
                            nc.tensor.matmul(
                                k_ps[mt][:],
                                wk_sb[:, ck, mt * P:(mt + 1) * P],
                                xk_t[sl][:, ck, :],
                                start=(ck == 0), stop=(ck == CK - 1))
                    for mt in range(2):
                        nc.vector.tensor_copy(kt_t[sl][:, mt, :], k_ps[mt][:])

                def fire(sl):
                    first = KSL_OFF[sl] // P
                    for st in range(first, first + KSL[sl] // P):
                        pt0[(0, st)] = st_step(0, 0, st)

                k_pass(0)        # xk slice 0 lands first; runs during xq DMA
                q_pass(0)
                fire(0)          # st 0-1
                k_pass(1)
                fire(1)          # st 2-3
                k_pass(2)
                fire(2)          # st 4-7
                k_pass(3)
                fire(3)          # st 8-11
                k_pass(4)
                fire(4)          # st 12-15
                q_pass(1)

            # =========== Phase B: V projection + pair-1 lch0 ST steps =======
            b_jobs = [(1, st) for st in range(16)]
            with tc.tile_pool(name="ps_v", bufs=4, space="PSUM") as psv:
                for q in range(4):
                    for half in range(2):
                        v_ps = [psv.tile([P, M], f32, tag="vv",
                                         name=f"vps{q}_{half}_{i}")
                                for i in range(2)]
                        for ck in range(CK):
                            for st2 in range(2):
                                st4 = half * 2 + st2
                                nc.tensor.matmul(
                                    v_ps[st2][:],
                                    xv_t[q][:, ck, st4 * P:(st4 + 1) * P],
                                    wv_sb[:, ck, :],
                                    start=(ck == 0), stop=(ck == CK - 1))
                            if ck in (3, 7) and b_jobs:
                                pair, st = b_jobs.pop(0)
                                pt0[(pair, st)] = st_step(0, pair, st)
                        for st2 in range(2):
                            st = q * 4 + half * 2 + st2
                            nc.vector.tensor_copy(
                                vones[:, st, :, 0:D],
                                v_ps[st2][:].rearrange("p (h d) -> p h d", h=HPC))
                for pair, st in b_jobs:
                    pt0[(pair, st)] = st_step(0, pair, st)

            # ====== Merged phase: per slot: O(lch0) + O(lch1) + ST(lch1).
            # pair1's O accumulations lag (3 and 5 slots) so the pair0 norms
            # have PE cover before pair1's matmuls alias pair0's PSUM banks.
            o_ps0 = {}   # pair -> [o_ps hh0, hh1] for lch0
            o_ps1 = {}
            # right side: independent pool stack, so pst (left) can close
            # before the norms finish and ps_wo reuses pst's banks.
            with tc.tile_pool(name="ps_o", bufs=4, space="PSUM",
                              side="right") as pso:
                def o_tiles(lch, pair):
                    return [pso.tile([D + 1, LCH], f32, tag="oo",
                                     name=f"ops{lch}_{pair}_{h}")
                            for h in range(2)]

                o_ps0[0] = o_tiles(0, 0)
                o_ps1[0] = o_tiles(1, 0)
                for s in range(NST):
                    # ST first: its kt weight loads hide behind the previous
                    # slot's O streams.  (pt pool rotation is 4 slots deep,
                    # so the ST write only depends on slot s-4's O reads.)
                    pt1[(0, s)] = st_step(1, 0, s)
                    o_step(o_ps0[0], s, 0, pt0.pop((0, s)))
                    if s >= 2:
                        o_step(o_ps1[0], s - 2, 0, pt1.pop((0, s - 2)))
                norm_lch(0, 0, o_ps0[0])
                o_ps0[1] = o_tiles(0, 1)
                o_ps1[1] = o_tiles(1, 1)
                for s in range(NST):
                    pt1[(1, s)] = st_step(1, 1, s)
                    if s < 2:
                        o_step(o_ps1[0], 14 + s, 0, pt1.pop((0, 14 + s)))
                    if s == 1:
                        norm_lch(0, 1, o_ps1[0])
                    if s >= 3:
                        o_step(o_ps0[1], s - 3, 1, pt0.pop((1, s - 3)))
                    if s >= 5:
                        o_step(o_ps1[1], s - 5, 1, pt1.pop((1, s - 5)))
                # drains + tail norms (Wo-lch0 unblocks after norm(1,0))
                for s in range(13, 16):
                    o_step(o_ps0[1], s, 1, pt0.pop((1, s)))
                norm_lch(1, 0, o_ps0[1], tail=True)
                for s in range(11, 16):
                    o_step(o_ps1[1], s, 1, pt1.pop((1, s)))
                norm_lch(1, 1, o_ps1[1], tail=True)

                pst_cm.__exit__(None, None, None)

                # ======= Wo tail: lch0 jobs first (unblocked earlier).
                # First 6 casts ride the scalar engine while the DVE finishes
                # the pair-1 norms; the rest alternate scalar/vector.
                with tc.tile_pool(name="ps_wo", bufs=4, space="PSUM") as pswo:
                    ncast = 0
                    for lt in range(8):
                        for nch in range(2):
                            eng = ("scalar" if ncast < 6 or ncast % 2 == 0
                                   else "vector")
                            wo_step(pswo, lt, nch, eng)
                            ncast += 1

    nc.compile()
    return nc


def _get_nc():
    if "nc" not in _cache:
        _cache["nc"] = _build()
    return _cache["nc"]


def _make_in_maps(inputs):
    import ml_dtypes

    bf16 = ml_dtypes.bfloat16
    query = np.asarray(inputs["query"], dtype=np.float32)
    key = np.asarray(inputs["key"], dtype=np.float32)
    value = np.asarray(inputs["value"], dtype=np.float32)
    Wq = np.asarray(inputs["Wq"], dtype=np.float32)
    Wk = np.asarray(inputs["Wk"], dtype=np.float32)
    Wv = np.asarray(inputs["Wv"], dtype=np.float32)
    Wo = np.asarray(inputs["Wo"], dtype=np.float32)

    qT = [np.ascontiguousarray(query[b].T).astype(bf16) for b in range(B)]
    kT = [np.ascontiguousarray(key[b].T).astype(bf16) for b in range(B)]
    vT = [np.ascontiguousarray(value[b].T).astype(bf16) for b in range(B)]
    wq_s = [np.ascontiguousarray(Wq[:, g * M:(g + 1) * M]).astype(bf16) for g in range(4)]
    wk_s = [np.ascontiguousarray(Wk[:, g * M:(g + 1) * M]).astype(bf16) for g in range(4)]
    wv_s = [np.ascontiguousarray(Wv[:, g * M:(g + 1) * M]).astype(bf16) for g in range(4)]
    wo_s = [np.ascontiguousarray(Wo[g * M:(g + 1) * M, :]).astype(bf16) for g in range(4)]

    in_maps = []
    for core in range(NCORES):
        b, g = core // 4, core % 4
        in_maps.append({
            "xqT": qT[b], "xkT": kT[b], "xvT": vT[b],
            "wq": wq_s[g], "wk": wk_s[g], "wv": wv_s[g], "wo": wo_s[g],
        })
    return in_maps


def kernel(query, key, value, Wq, Wk, Wv, Wo, bo):
    from concourse.bass_utils import run_bass_kernel_spmd

    nc = _get_nc()
    bo = np.asarray(bo, dtype=np.float32)
    in_maps = _make_in_maps(dict(query=query, key=key, value=value,
                                 Wq=Wq, Wk=Wk, Wv=Wv, Wo=Wo))

    res = run_bass_kernel_spmd(nc, in_maps, core_ids=list(range(NCORES)))

    out = np.zeros((B, L, C), dtype=np.float32)
    for core in range(NCORES):
        b = core // 4
        out[b] += np.asarray(res.results[core]["outp"], dtype=np.float32)
    out += bo[None, None, :]
    return out


# revision 26
# speedup vs baseline: 1.0488x; 1.0188x over previous
"""Trainium2 Bass kernel for nn_CrossAttention (B=2, L=1024, S=2048, DIM=1024, H=16 heads).

Sharding: tensor-parallel over heads x data-parallel over batch.
Core c handles batch b = c//4 and head-group g = c%4 (4 heads = 256 of the
1024 hidden channels).  Each core computes, for its (b, g):

    QT = (Wq_g)^T x_q^T          [256, 1024]   (m on partitions)
    KT = (Wk_g)^T x_k^T          [256, 2048]
    V  = x_v Wv_g                [2048, 256]   (s on partitions)
    per head h (d=64):
        ST_h = KT_h^T' ...       S^T[s, l] = k_s . q_l   (s on partitions)
        P_h  = exp(SCALE * ST_h)            (unnormalized, s on partitions)
        [O^T_h ; sums_h] = [V_h | 1]^T @ P_h   (ones-column folds the softmax
                                                denominator into the matmul)
        XgT_h = O^T_h * (1/sums_h)          (gpsimd partition broadcast)
    out_partial = XgT^T @ Wo_g   [1024, 1024]

Host gathers: out[b] = sum_g out_partial[4b+g] + bo.

v3 schedule: the kernel is jointly PE- and ACT(exp)-bound; the structure
minimizes time-to-first-exp, keeps the 64-exp stream dense, and minimizes
the serial tail after the last exp:
  - DMAs only move from ~8.7us (runtime preamble), so the pre-exp bytes are
    cut to 2.5MB: wq, xq l-half 0, wk, first 256-wide xk slice.  xk arrives
    as 5 s-slices so ST steps unlock progressively; xq l-half 1 is deferred
    behind the first three xk slices (lch1 STs don't run until the merged
    phase).  qt/kt/xgt live as per-region tiles so readers never see
    false whole-tile dependencies on later writers.
  - ~20 warmup matmuls keep the PE HAM clock-gate busy (2.4GHz) until real
    data lands; phase A then runs DMA-paced with pair-0 ST steps fired as
    each kt slice is cast.  Phase B (V projection) paces the pair-1 steps.
  - Merged phase: per slot: O(lch0) + O(lch1, lagged) + ST(lch1); pair1's
    O accumulations lag 3/5 slots so the pair0 norms get PE cover before
    pair1's matmuls alias pair0's PSUM banks.  Both O accumulations finish
    with the exp stream, so the post-exp tail is only norm + Wo.
  - Wo PSUM opens after the ST pool closes (its banks are gated only by the
    last exp reads, not the norms); jobs run lch0-first, casts alternate
    scalar/vector engines.
Output partials are written bf16 (host accumulates fp32); the softmax
reciprocal uses the fast approximate DVE op + gpsimd partition broadcast.
"""

import sys

if "/opt/trn_rl_repo" not in sys.path:
    sys.path.insert(0, "/opt/trn_rl_repo")

import numpy as np

B, L, S, C = 2, 1024, 2048, 1024
NH, D = 16, 64          # total heads, head dim
HPC = 4                 # heads per core
M = HPC * D             # 256 output channels per core
SCALE = D ** -0.5
P = 128                 # partitions
NCORES = 8
CK = C // P             # 8 c-tiles
NST = S // P            # 16 s-tiles
LCH = 512               # l-chunk
NLCH = L // LCH         # 2

# xk s-slice widths (elements of s); first two narrow so the first ST
# steps unlock with minimal DMA.
KSL = [256, 256, 512, 512, 512]
KSL_OFF = [0, 256, 512, 1024, 1536]
# st tile -> (slice index, offset within slice)
ST_SLICE = []
for _i, (_o, _w) in enumerate(zip(KSL_OFF, KSL)):
    for _j in range(_w // P):
        ST_SLICE.append((_i, _j * P))

_cache = {}


def _build():
    import concourse.tile as tile
    from concourse import mybir, bacc

    f32 = mybir.dt.float32
    bf16 = mybir.dt.bfloat16

    nc = bacc.Bacc("TRN2", target_bir_lowering=False, debug=False)

    xqT = nc.dram_tensor("xqT", [C, L], bf16, kind="ExternalInput")
    xkT = nc.dram_tensor("xkT", [C, S], bf16, kind="ExternalInput")
    xvT = nc.dram_tensor("xvT", [C, S], bf16, kind="ExternalInput")
    wq = nc.dram_tensor("wq", [C, M], bf16, kind="ExternalInput")
    wk = nc.dram_tensor("wk", [C, M], bf16, kind="ExternalInput")
    wv = nc.dram_tensor("wv", [C, M], bf16, kind="ExternalInput")
    wo = nc.dram_tensor("wo", [M, C], bf16, kind="ExternalInput")
    outp = nc.dram_tensor("outp", [L, C], bf16, kind="ExternalOutput")

    with tile.TileContext(nc) as tc:
        with tc.tile_pool(name="singles", bufs=1) as singles, \
             tc.tile_pool(name="xk_pool", bufs=3) as xkp, \
             tc.tile_pool(name="xv_pool", bufs=3) as xvp, \
             tc.tile_pool(name="pts", bufs=36) as pts, \
             tc.tile_pool(name="small", bufs=3) as small, \
             tc.tile_pool(name="obuf", bufs=4) as obuf:

            # ---- persistent SBUF ----
            wq_sb = singles.tile([P, CK, M], bf16, tag="wq")
            wk_sb = singles.tile([P, CK, M], bf16, tag="wk")
            wv_sb = singles.tile([P, CK, M], bf16, tag="wv")
            wo_sb = singles.tile([P, M // P, C], bf16, tag="wo")
            xq_sb = singles.tile([P, CK, L], bf16, tag="xq")

            # DMA priority order = consumption order: the first ST+exp needs
            # only wk + xk slice 0 + wq + xq l-half 0 (2.5MB).
            xk_t = []

            def load_xk(i):
                t = xkp.tile([P, CK, KSL[i]], bf16, tag="xk", name=f"xk{i}")
                nc.sync.dma_start(
                    t[:], xkT[:, KSL_OFF[i]:KSL_OFF[i] + KSL[i]]
                    .rearrange("(ck p) s -> p ck s", p=P))
                xk_t.append(t)

            xv_t = []

            def load_xv(q):
                t = xvp.tile([P, CK, 4 * P], bf16, tag="xv", name=f"xv{q}")
                nc.sync.dma_start(
                    t[:], xvT[:, q * 4 * P:(q + 1) * 4 * P]
                    .rearrange("(ck p) s -> p ck s", p=P))
                xv_t.append(t)

            nc.sync.dma_start(wk_sb[:], wk.rearrange("(ck p) m -> p ck m", p=P))
            load_xk(0)
            nc.sync.dma_start(wq_sb[:], wq.rearrange("(ck p) m -> p ck m", p=P))
            nc.sync.dma_start(xq_sb[:, :, 0:LCH],
                              xqT[:, 0:LCH].rearrange("(ck p) l -> p ck l", p=P))
            load_xk(1)
            load_xk(2)
            load_xk(3)
            load_xk(4)
            nc.sync.dma_start(wv_sb[:], wv.rearrange("(ck p) m -> p ck m", p=P))
            load_xv(0)
            load_xv(1)
            # xq l-half 1 is only needed for the lch1 STs (merged phase)
            nc.sync.dma_start(xq_sb[:, :, LCH:L],
                              xqT[:, LCH:L].rearrange("(ck p) l -> p ck l", p=P))
            load_xv(2)
            load_xv(3)
            nc.sync.dma_start(wo_sb[:], wo.rearrange("(kt p) n -> p kt n", p=P))

            # qt/kt/xgt as per-region tiles (no false whole-tile deps)
            qt_t = [singles.tile([P, 2, LCH], bf16, tag=f"qt{lh}",
                                 name=f"qt{lh}")
                    for lh in range(2)]                    # [m%128, m//128, l]
            kt_t = [singles.tile([P, 2, KSL[i]], bf16, tag=f"kt{i}",
                                 name=f"kt{i}")
                    for i in range(5)]                     # [m%128, m//128, s]
            vones = singles.tile([P, NST, HPC, D + 1], bf16, tag="vones")
            xgt_t = [singles.tile([P, 2, LCH], bf16, tag=f"xgt{lc}",
                                  name=f"xgt{lc}")
                     for lc in range(2)]
            stage = singles.tile([P, D], f32, tag="stage")
            nc.vector.memset(stage[:], 1.0)
            nc.vector.tensor_copy(vones[:, :, :, D:D + 1],
                                  stage[:].rearrange("p (a b) -> p a b", a=NST)[:, :, :, None])

            # ---- step helpers ----
            def st_step(lch, pair, st):
                """ST pair matmuls + exp; returns the PT tile."""
                sl, so = ST_SLICE[st]
                st_ps = pst.tile([P, 2, LCH], f32, tag="st", name=f"stps_{lch}_{pair}_{st}")
                nc.tensor.matmul(
                    st_ps[:, 0, :], kt_t[sl][0:D, pair, so:so + P],
                    qt_t[lch][0:D, pair, :], start=True, stop=True)
                nc.tensor.matmul(
                    st_ps[:, 1, :], kt_t[sl][D:P, pair, so:so + P],
                    qt_t[lch][D:P, pair, :], start=True, stop=True,
                    tile_position=(64, 0))
                pt_t = pts.tile([P, 2, LCH], bf16, tag="pt", name=f"pt_{lch}_{pair}_{st}")
                nc.scalar.activation(pt_t[:], st_ps[:],
                                     mybir.ActivationFunctionType.Exp, scale=SCALE)
                return pt_t

            def o_step(o_ps, st, pair, pt_t):
                for hh in range(2):
                    nc.tensor.matmul(
                        o_ps[hh][:], vones[:, st, pair * 2 + hh, :], pt_t[:, hh, :],
                        start=(st == 0), stop=(st == NST - 1))

            def norm_lch(pair, lch, o_ps, tail=False):
                """per-(pair,lch) softmax normalize: fast reciprocal of the
                sums row -> gpsimd partition broadcast -> scaled XgT."""
                for hh in range(2):
                    sums_sb = small.tile([1, LCH], f32, tag="sums")
                    if tail:
                        # scalar engine is idle post-exp; keep the DVE free
                        # for the reciprocals/multiplies on the critical tail
                        nc.scalar.copy(sums_sb[:], o_ps[hh][D:D + 1, :])
                    else:
                        nc.vector.tensor_copy(sums_sb[:], o_ps[hh][D:D + 1, :])
                    rc = small.tile([1, LCH], f32, tag="rc")
                    nc.vector.reciprocal_approx_fast(rc[:], sums_sb[:])
                    bc_sb = small.tile([D, LCH], f32, tag="bc")
                    nc.gpsimd.partition_broadcast(bc_sb[:], rc[:])
                    nc.vector.tensor_mul(
                        xgt_t[lch][hh * D:(hh + 1) * D, pair, :],
                        o_ps[hh][0:D, :], bc_sb[:])

            def wo_step(pool, lt, nch, cast_eng):
                wo_ps = pool.tile([P, 512], f32, tag="wo", name=f"wops_{lt}_{nch}")
                for kt in range(2):
                    nc.tensor.matmul(
                        wo_ps[:], xgt_t[lt // 4][:, kt, (lt % 4) * P:(lt % 4 + 1) * P],
                        wo_sb[:, kt, nch * 512:(nch + 1) * 512],
                        start=(kt == 0), stop=(kt == 1))
                ob_sb = obuf.tile([P, 512], bf16, tag="ob")
                if cast_eng == "scalar":
                    nc.scalar.copy(ob_sb[:], wo_ps[:])
                else:
                    nc.vector.tensor_copy(ob_sb[:], wo_ps[:])
                # sync engine is idle at the tail; its DMAs stripe 16 queues
                nc.sync.dma_start(
                    outp[lt * P:(lt + 1) * P, nch * 512:(nch + 1) * 512], ob_sb[:])

            # ---- PSUM pool timeline:
            #   pst(4) > [psw(1) warmup] > [psp(4) A] > [psv(4) B] >
            #   [pso(4) merged]; pst closes after the last ST, then ps_wo(4)
            #   reuses pst's banks (gated only by the last exp reads).
            pst_cm = tc.tile_pool(name="ps_st", bufs=2, space="PSUM")
            pst = pst_cm.__enter__()

            pt0 = {}   # (pair, st) -> PT tile for lch 0
            pt1 = {}

            # PE warm-up: keep the HAM clock-gate busy until real data lands.
            warm_sb = singles.tile([P, 512], bf16, tag="warm")
            nc.vector.memset(warm_sb[:], 1.0)
            with tc.tile_pool(name="ps_warm", bufs=1, space="PSUM") as psw:
                warm = psw.tile([P, 512], f32, tag="warm")
                for i in range(8):
                    nc.tensor.matmul(
                        warm[:], warm_sb[:, 0:128], warm_sb[:],
                        start=True, stop=True)

            # =========== Phase A: projections + pair-0 ST steps =============
            with tc.tile_pool(name="ps_proj", bufs=4, space="PSUM") as psp:

                def q_pass(lh):
                    # mt-sequential: pair-0 consumers only need mt 0
                    for mt in range(2):
                        q_ps = psp.tile([P, 512], f32, tag="pp",
                                        name=f"qtps{lh}_{mt}")
                        for ck in range(CK):
                            nc.tensor.matmul(
                                q_ps[:],
                                wq_sb[:, ck, mt * P:(mt + 1) * P],
                                xq_sb[:, ck, lh * 512:(lh + 1) * 512],
                                start=(ck == 0), stop=(ck == CK - 1))
                        nc.vector.tensor_copy(qt_t[lh][:, mt, :], q_ps[:])

                def fire(sl):
                    first = KSL_OFF[sl] // P
                    for st in range(first, first + KSL[sl] // P):
                        pt0[(0, st)] = st_step(0, 0, st)

                def k_pass(sl, fire_sts=True):
                    # mt 0 chain + cast first, fire its pair-0 STs, then mt 1
                    w = KSL[sl]
                    for mt in range(2):
                        k_ps = psp.tile([P, w], f32, tag="pp",
                                        name=f"ktps{sl}_{mt}")
                        for ck in range(CK):
                            nc.tensor.matmul(
                                k_ps[:],
                                wk_sb[:, ck, mt * P:(mt + 1) * P],
                                xk_t[sl][:, ck, :],
                                start=(ck == 0), stop=(ck == CK - 1))
                        nc.vector.tensor_copy(kt_t[sl][:, mt, :], k_ps[:])
                        if mt == 0 and fire_sts:
                            fire(sl)

                k_pass(0, fire_sts=False)   # xk slice 0 lands first (no qt yet)
                q_pass(0)
                fire(0)
                for sl in range(1, 5):
                    k_pass(sl)

            # =========== Phase B: V projection + pair-1 lch0 ST steps =======
            b_jobs = [(1, st) for st in range(16)]
            with tc.tile_pool(name="ps_v", bufs=4, space="PSUM") as psv:
                def q_pass_b(lh):
                    for mt in range(2):
                        q_ps = psv.tile([P, 512], f32, tag="vv",
                                        name=f"qtpsb{lh}_{mt}")
                        for ck in range(CK):
                            nc.tensor.matmul(
                                q_ps[:],
                                wq_sb[:, ck, mt * P:(mt + 1) * P],
                                xq_sb[:, ck, lh * 512:(lh + 1) * 512],
                                start=(ck == 0), stop=(ck == CK - 1))
                        nc.vector.tensor_copy(qt_t[lh][:, mt, :], q_ps[:])

                for q in range(4):
                    if q == 1:
                        q_pass_b(1)   # lch1 qt, needed from the merged phase
                    for half in range(2):
                        v_ps = [psv.tile([P, M], f32, tag="vv",
                                         name=f"vps{q}_{half}_{i}")
                                for i in range(2)]
                        for ck in range(CK):
                            for st2 in range(2):
                                st4 = half * 2 + st2
                                nc.tensor.matmul(
                                    v_ps[st2][:],
                                    xv_t[q][:, ck, st4 * P:(st4 + 1) * P],
                                    wv_sb[:, ck, :],
                                    start=(ck == 0), stop=(ck == CK - 1))
                            if ck in (3, 7) and b_jobs:
                                pair, st = b_jobs.pop(0)
                                pt0[(pair, st)] = st_step(0, pair, st)
                        for st2 in range(2):
                            st = q * 4 + half * 2 + st2
                            nc.vector.tensor_copy(
                                vones[:, st, :, 0:D],
                                v_ps[st2][:].rearrange("p (h d) -> p h d", h=HPC))
                for pair, st in b_jobs:
                    pt0[(pair, st)] = st_step(0, pair, st)

            # ====== Merged phase: per slot: O(lch0) + O(lch1) + ST(lch1).
            # pair1's O accumulations lag (3 and 5 slots) so the pair0 norms
            # have PE cover before pair1's matmuls alias pair0's PSUM banks.
            o_ps0 = {}   # pair -> [o_ps hh0, hh1] for lch0
            o_ps1 = {}
            # right side: independent pool stack, so pst (left) can close
            # before the norms finish and ps_wo reuses pst's banks.
            with tc.tile_pool(name="ps_o", bufs=4, space="PSUM",
                              side="right") as pso:
                def o_tiles(lch, pair):
                    return [pso.tile([D + 1, LCH], f32, tag="oo",
                                     name=f"ops{lch}_{pair}_{h}")
                            for h in range(2)]

                o_ps0[0] = o_tiles(0, 0)
                o_ps1[0] = o_tiles(1, 0)
                for s in range(NST):
                    # ST mid-slot: its kt weight loads hide behind O(lch0)'s
                    # streams.  (pt pool rotation is 4 slots deep, so the ST
                    # write only depends on slot s-4's O reads.)
                    o_step(o_ps0[0], s, 0, pt0.pop((0, s)))
                    pt1[(0, s)] = st_step(1, 0, s)
                    if s >= 2:
                        o_step(o_ps1[0], s - 2, 0, pt1.pop((0, s - 2)))
                norm_lch(0, 0, o_ps0[0])
                o_ps0[1] = o_tiles(0, 1)
                o_ps1[1] = o_tiles(1, 1)
                for s in range(NST):
                    if s < 2:
                        o_step(o_ps1[0], 14 + s, 0, pt1.pop((0, 14 + s)))
                    if s == 1:
                        norm_lch(0, 1, o_ps1[0])
                    if s >= 3:
                        o_step(o_ps0[1], s - 3, 1, pt0.pop((1, s - 3)))
                    pt1[(1, s)] = st_step(1, 1, s)
                    if s >= 5:
                        o_step(o_ps1[1], s - 5, 1, pt1.pop((1, s - 5)))
                # drains + tail norms (Wo-lch0 unblocks after norm(1,0))
                for s in range(13, 16):
                    o_step(o_ps0[1], s, 1, pt0.pop((1, s)))
                norm_lch(1, 0, o_ps0[1], tail=True)
                for s in range(11, 16):
                    o_step(o_ps1[1], s, 1, pt1.pop((1, s)))
                norm_lch(1, 1, o_ps1[1], tail=True)

                pst_cm.__exit__(None, None, None)

                # ======= Wo tail: lch0 jobs first (unblocked earlier) =======
                with tc.tile_pool(name="ps_wo", bufs=4, space="PSUM") as pswo:
                    ncast = 0
                    for lt in range(8):
                        for nch in range(2):
                            wo_step(pswo, lt, nch,
                                    "scalar" if ncast % 2 == 0 else "vector")
                            ncast += 1

    nc.compile()
    return nc


def _get_nc():
    if "nc" not in _cache:
        _cache["nc"] = _build()
    return _cache["nc"]


def _make_in_maps(inputs):
    import ml_dtypes

    bf16 = ml_dtypes.bfloat16
    query = np.asarray(inputs["query"], dtype=np.float32)
    key = np.asarray(inputs["key"], dtype=np.float32)
    value = np.asarray(inputs["value"], dtype=np.float32)
    Wq = np.asarray(inputs["Wq"], dtype=np.float32)
    Wk = np.asarray(inputs["Wk"], dtype=np.float32)
    Wv = np.asarray(inputs["Wv"], dtype=np.float32)
    Wo = np.asarray(inputs["Wo"], dtype=np.float32)

    qT = [np.ascontiguousarray(query[b].T).astype(bf16) for b in range(B)]
    kT = [np.ascontiguousarray(key[b].T).astype(bf16) for b in range(B)]
    vT = [np.ascontiguousarray(value[b].T).astype(bf16) for b in range(B)]
    wq_s = [np.ascontiguousarray(Wq[:, g * M:(g + 1) * M]).astype(bf16) for g in range(4)]
    wk_s = [np.ascontiguousarray(Wk[:, g * M:(g + 1) * M]).astype(bf16) for g in range(4)]
    wv_s = [np.ascontiguousarray(Wv[:, g * M:(g + 1) * M]).astype(bf16) for g in range(4)]
    wo_s = [np.ascontiguousarray(Wo[g * M:(g + 1) * M, :]).astype(bf16) for g in range(4)]

    in_maps = []
    for core in range(NCORES):
        b, g = core // 4, core % 4
        in_maps.append({
            "xqT": qT[b], "xkT": kT[b], "xvT": vT[b],
            "wq": wq_s[g], "wk": wk_s[g], "wv": wv_s[g], "wo": wo_s[g],
        })
    return in_maps


def kernel(query, key, value, Wq, Wk, Wv, Wo, bo):
    from concourse.bass_utils import run_bass_kernel_spmd

    nc = _get_nc()
    bo = np.asarray(bo, dtype=np.float32)
    in_maps = _make_in_maps(dict(query=query, key=key, value=value,
                                 Wq=Wq, Wk=Wk, Wv=Wv, Wo=Wo))

    res = run_bass_kernel_spmd(nc, in_maps, core_ids=list(range(NCORES)))

    out = np.zeros((B, L, C), dtype=np.float32)
    for core in range(NCORES):
        b = core // 4
        out[b] += np.asarray(res.results[core]["outp"], dtype=np.float32)
    out += bo[None, None, :]
    return out


# revision 30
# speedup vs baseline: 1.0509x; 1.0020x over previous
"""Trainium2 Bass kernel for nn_CrossAttention (B=2, L=1024, S=2048, DIM=1024, H=16 heads).

Sharding: tensor-parallel over heads x data-parallel over batch.
Core c handles batch b = c//4 and head-group g = c%4 (4 heads = 256 of the
1024 hidden channels).  Each core computes, for its (b, g):

    QT = (Wq_g)^T x_q^T          [256, 1024]   (m on partitions)
    KT = (Wk_g)^T x_k^T          [256, 2048]
    V  = x_v Wv_g                [2048, 256]   (s on partitions)
    per head h (d=64):
        ST_h = KT_h^T' ...       S^T[s, l] = k_s . q_l   (s on partitions)
        P_h  = exp(SCALE * ST_h)            (unnormalized, s on partitions)
        [O^T_h ; sums_h] = [V_h | 1]^T @ P_h   (ones-column folds the softmax
                                                denominator into the matmul)
        XgT_h = O^T_h * (1/sums_h)          (gpsimd partition broadcast)
    out_partial = XgT^T @ Wo_g   [1024, 1024]

Host gathers: out[b] = sum_g out_partial[4b+g] + bo.

v3 schedule: the kernel is jointly PE- and ACT(exp)-bound; the structure
minimizes time-to-first-exp, keeps the 64-exp stream dense, and minimizes
the serial tail after the last exp:
  - DMAs only move from ~8.7us (runtime preamble), so the pre-exp bytes are
    cut to 2.5MB: wq, xq l-half 0, wk, first 256-wide xk slice.  xk arrives
    as 5 s-slices so ST steps unlock progressively; xq l-half 1 is deferred
    behind the first three xk slices (lch1 STs don't run until the merged
    phase).  qt/kt/xgt live as per-region tiles so readers never see
    false whole-tile dependencies on later writers.
  - ~20 warmup matmuls keep the PE HAM clock-gate busy (2.4GHz) until real
    data lands; phase A then runs DMA-paced with pair-0 ST steps fired as
    each kt slice is cast.  Phase B (V projection) paces the pair-1 steps.
  - Merged phase: per slot: O(lch0) + O(lch1, lagged) + ST(lch1); pair1's
    O accumulations lag 3/5 slots so the pair0 norms get PE cover before
    pair1's matmuls alias pair0's PSUM banks.  Both O accumulations finish
    with the exp stream, so the post-exp tail is only norm + Wo.
  - Wo PSUM opens after the ST pool closes (its banks are gated only by the
    last exp reads, not the norms); jobs run lch0-first, casts alternate
    scalar/vector engines.
Output partials are written bf16 (host accumulates fp32); the softmax
reciprocal uses the fast approximate DVE op + gpsimd partition broadcast.
"""

import sys

if "/opt/trn_rl_repo" not in sys.path:
    sys.path.insert(0, "/opt/trn_rl_repo")

import numpy as np

B, L, S, C = 2, 1024, 2048, 1024
NH, D = 16, 64          # total heads, head dim
HPC = 4                 # heads per core
M = HPC * D             # 256 output channels per core
SCALE = D ** -0.5
P = 128                 # partitions
NCORES = 8
CK = C // P             # 8 c-tiles
NST = S // P            # 16 s-tiles
LCH = 512               # l-chunk
NLCH = L // LCH         # 2

# xk s-slice widths (elements of s); first two narrow so the first ST
# steps unlock with minimal DMA.
KSL = [256, 256, 512, 512, 512]
KSL_OFF = [0, 256, 512, 1024, 1536]
# st tile -> (slice index, offset within slice)
ST_SLICE = []
for _i, (_o, _w) in enumerate(zip(KSL_OFF, KSL)):
    for _j in range(_w // P):
        ST_SLICE.append((_i, _j * P))

_cache = {}


def _build():
    import concourse.tile as tile
    from concourse import mybir, bacc

    f32 = mybir.dt.float32
    bf16 = mybir.dt.bfloat16

    nc = bacc.Bacc("TRN2", target_bir_lowering=False, debug=False)

    # inputs are pre-tiled on the host into per-partition-contiguous SBUF
    # layouts so every DMA moves 4-8KB contiguous chunks per partition
    xqT = nc.dram_tensor("xqT", [P, 2, CK, LCH], bf16, kind="ExternalInput")
    xkT = nc.dram_tensor("xkT", [P, CK * S], bf16, kind="ExternalInput")
    xvT = nc.dram_tensor("xvT", [P, CK * S], bf16, kind="ExternalInput")
    wq = nc.dram_tensor("wq", [P, CK, M], bf16, kind="ExternalInput")
    wk = nc.dram_tensor("wk", [P, CK, M], bf16, kind="ExternalInput")
    wv = nc.dram_tensor("wv", [P, CK, M], bf16, kind="ExternalInput")
    wo = nc.dram_tensor("wo", [P, M // P, C], bf16, kind="ExternalInput")
    outp = nc.dram_tensor("outp", [L, C], bf16, kind="ExternalOutput")

    with tile.TileContext(nc) as tc:
        with tc.tile_pool(name="singles", bufs=1) as singles, \
             tc.tile_pool(name="xk_pool", bufs=3) as xkp, \
             tc.tile_pool(name="xv_pool", bufs=3) as xvp, \
             tc.tile_pool(name="pts", bufs=36) as pts, \
             tc.tile_pool(name="small", bufs=3) as small, \
             tc.tile_pool(name="obuf", bufs=6) as obuf:

            # ---- persistent SBUF ----
            wq_sb = singles.tile([P, CK, M], bf16, tag="wq")
            wk_sb = singles.tile([P, CK, M], bf16, tag="wk")
            wv_sb = singles.tile([P, CK, M], bf16, tag="wv")
            wo_sb = singles.tile([P, M // P, C], bf16, tag="wo")
            xq_sb = singles.tile([P, 2, CK, LCH], bf16, tag="xq")

            # DMA priority order = consumption order: the first ST+exp needs
            # only wk + xk slice 0 + wq + xq l-half 0 (2.5MB).
            xk_t = []

            def load_xk(i):
                t = xkp.tile([P, CK, KSL[i]], bf16, tag="xk", name=f"xk{i}")
                base = CK * KSL_OFF[i]
                nc.sync.dma_start(
                    t[:], xkT[:, base:base + CK * KSL[i]]
                    .rearrange("p (ck w) -> p ck w", ck=CK))
                xk_t.append(t)

            xv_t = []

            def load_xv(q):
                t = xvp.tile([P, CK, 4 * P], bf16, tag="xv", name=f"xv{q}")
                base = CK * 4 * P * q
                nc.sync.dma_start(
                    t[:], xvT[:, base:base + CK * 4 * P]
                    .rearrange("p (ck w) -> p ck w", ck=CK))
                xv_t.append(t)

            nc.sync.dma_start(wk_sb[:], wk[:])
            load_xk(0)
            nc.sync.dma_start(wq_sb[:], wq[:])
            nc.sync.dma_start(xq_sb[:, 0], xqT[:, 0])
            load_xk(1)
            load_xk(2)
            load_xk(3)
            load_xk(4)
            nc.sync.dma_start(wv_sb[:], wv[:])
            load_xv(0)
            load_xv(1)
            # xq l-half 1 is only needed for the lch1 STs (merged phase)
            nc.sync.dma_start(xq_sb[:, 1], xqT[:, 1])
            load_xv(2)
            load_xv(3)
            nc.sync.dma_start(wo_sb[:], wo[:])

            # qt/kt/xgt as per-region tiles (no false whole-tile deps)
            qt_t = [singles.tile([P, 2, LCH], bf16, tag=f"qt{lh}",
                                 name=f"qt{lh}")
                    for lh in range(2)]                    # [m%128, m//128, l]
            kt_t = [singles.tile([P, 2, KSL[i]], bf16, tag=f"kt{i}",
                                 name=f"kt{i}")
                    for i in range(5)]                     # [m%128, m//128, s]
            vones = singles.tile([P, NST, HPC, D + 1], bf16, tag="vones")
            xgt_t = [singles.tile([P, 2, LCH], bf16, tag=f"xgt{lc}",
                                  name=f"xgt{lc}")
                     for lc in range(2)]
            stage = singles.tile([P, D], f32, tag="stage")
            nc.vector.memset(stage[:], 1.0)
            nc.vector.tensor_copy(vones[:, :, :, D:D + 1],
                                  stage[:].rearrange("p (a b) -> p a b", a=NST)[:, :, :, None])

            # ---- step helpers ----
            def st_step(lch, pair, st):
                """ST pair matmuls + exp; returns the PT tile."""
                sl, so = ST_SLICE[st]
                st_ps = pst.tile([P, 2, LCH], f32, tag="st", name=f"stps_{lch}_{pair}_{st}")
                nc.tensor.matmul(
                    st_ps[:, 0, :], kt_t[sl][0:D, pair, so:so + P],
                    qt_t[lch][0:D, pair, :], start=True, stop=True)
                nc.tensor.matmul(
                    st_ps[:, 1, :], kt_t[sl][D:P, pair, so:so + P],
                    qt_t[lch][D:P, pair, :], start=True, stop=True,
                    tile_position=(64, 0))
                pt_t = pts.tile([P, 2, LCH], bf16, tag="pt", name=f"pt_{lch}_{pair}_{st}")
                nc.scalar.activation(pt_t[:], st_ps[:],
                                     mybir.ActivationFunctionType.Exp, scale=SCALE)
                return pt_t

            def o_step(o_ps, st, pair, pt_t):
                for hh in range(2):
                    nc.tensor.matmul(
                        o_ps[hh][:], vones[:, st, pair * 2 + hh, :], pt_t[:, hh, :],
                        start=(st == 0), stop=(st == NST - 1))

            def norm_lch(pair, lch, o_ps, tail=False):
                """per-(pair,lch) softmax normalize: fast reciprocal of the
                sums row -> gpsimd partition broadcast -> scaled XgT."""
                for hh in range(2):
                    sums_sb = small.tile([1, LCH], f32, tag="sums")
                    if tail:
                        # scalar engine is idle post-exp; keep the DVE free
                        # for the reciprocals/multiplies on the critical tail
                        nc.scalar.copy(sums_sb[:], o_ps[hh][D:D + 1, :])
                    else:
                        nc.vector.tensor_copy(sums_sb[:], o_ps[hh][D:D + 1, :])
                    rc = small.tile([1, LCH], f32, tag="rc")
                    nc.vector.reciprocal_approx_fast(rc[:], sums_sb[:])
                    rcb = small.tile([1, LCH], bf16, tag="rcb")
                    nc.vector.tensor_copy(rcb[:], rc[:])
                    bc_sb = small.tile([D, LCH], bf16, tag="bc")
                    nc.gpsimd.partition_broadcast(bc_sb[:], rcb[:])
                    nc.vector.tensor_mul(
                        xgt_t[lch][hh * D:(hh + 1) * D, pair, :],
                        o_ps[hh][0:D, :], bc_sb[:])

            def wo_step(pool, lt, nch, cast_eng):
                wo_ps = pool.tile([P, 512], f32, tag="wo", name=f"wops_{lt}_{nch}")
                for kt in range(2):
                    nc.tensor.matmul(
                        wo_ps[:], xgt_t[lt // 4][:, kt, (lt % 4) * P:(lt % 4 + 1) * P],
                        wo_sb[:, kt, nch * 512:(nch + 1) * 512],
                        start=(kt == 0), stop=(kt == 1))
                ob_sb = obuf.tile([P, 512], bf16, tag="ob")
                if cast_eng == "scalar":
                    nc.scalar.copy(ob_sb[:], wo_ps[:])
                else:
                    nc.vector.tensor_copy(ob_sb[:], wo_ps[:])
                # sync engine is idle at the tail; its DMAs stripe 16 queues
                nc.sync.dma_start(
                    outp[lt * P:(lt + 1) * P, nch * 512:(nch + 1) * 512], ob_sb[:])

            # ---- PSUM pool timeline:
            #   pst(4) > [psw(1) warmup] > [psp(4) A] > [psv(4) B] >
            #   [pso(4) merged]; pst closes after the last ST, then ps_wo(4)
            #   reuses pst's banks (gated only by the last exp reads).
            pst_cm = tc.tile_pool(name="ps_st", bufs=2, space="PSUM")
            pst = pst_cm.__enter__()

            pt0 = {}   # (pair, st) -> PT tile for lch 0
            pt1 = {}

            # PE warm-up: keep the HAM clock-gate busy until real data lands.
            warm_sb = singles.tile([P, 512], bf16, tag="warm")
            nc.vector.memset(warm_sb[:], 1.0)
            with tc.tile_pool(name="ps_warm", bufs=1, space="PSUM") as psw:
                warm = psw.tile([P, 512], f32, tag="warm")
                for i in range(8):
                    nc.tensor.matmul(
                        warm[:], warm_sb[:, 0:128], warm_sb[:],
                        start=True, stop=True)

            # =========== Phase A: projections + pair-0 ST steps =============
            with tc.tile_pool(name="ps_proj", bufs=4, space="PSUM") as psp:

                def q_pass(lh):
                    # mt-sequential: pair-0 consumers only need mt 0
                    for mt in range(2):
                        q_ps = psp.tile([P, 512], f32, tag="pp",
                                        name=f"qtps{lh}_{mt}")
                        for ck in range(CK):
                            nc.tensor.matmul(
                                q_ps[:],
                                wq_sb[:, ck, mt * P:(mt + 1) * P],
                                xq_sb[:, lh, ck, :],
                                start=(ck == 0), stop=(ck == CK - 1))
                        nc.vector.tensor_copy(qt_t[lh][:, mt, :], q_ps[:])

                def fire(sl):
                    first = KSL_OFF[sl] // P
                    for st in range(first, first + KSL[sl] // P):
                        pt0[(0, st)] = st_step(0, 0, st)

                def k_pass(sl, fire_sts=True):
                    # mt 0 chain + cast first, fire its pair-0 STs, then mt 1
                    w = KSL[sl]
                    for mt in range(2):
                        k_ps = psp.tile([P, w], f32, tag="pp",
                                        name=f"ktps{sl}_{mt}")
                        for ck in range(CK):
                            nc.tensor.matmul(
                                k_ps[:],
                                wk_sb[:, ck, mt * P:(mt + 1) * P],
                                xk_t[sl][:, ck, :],
                                start=(ck == 0), stop=(ck == CK - 1))
                        nc.vector.tensor_copy(kt_t[sl][:, mt, :], k_ps[:])
                        if mt == 0 and fire_sts:
                            fire(sl)

                k_pass(0, fire_sts=False)   # xk slice 0 lands first (no qt yet)
                q_pass(0)
                fire(0)
                for sl in range(1, 5):
                    k_pass(sl)

            # =========== Phase B: V projection + pair-1 lch0 ST steps =======
            b_jobs = [(1, st) for st in range(16)]
            with tc.tile_pool(name="ps_v", bufs=4, space="PSUM") as psv:
                def q_pass_b(lh, mt):
                    q_ps = psv.tile([P, 512], f32, tag="vv",
                                    name=f"qtpsb{lh}_{mt}")
                    for ck in range(CK):
                        nc.tensor.matmul(
                            q_ps[:],
                            wq_sb[:, ck, mt * P:(mt + 1) * P],
                            xq_sb[:, lh, ck, :],
                            start=(ck == 0), stop=(ck == CK - 1))
                    nc.vector.tensor_copy(qt_t[lh][:, mt, :], q_ps[:])

                for q in range(4):
                    if q in (1, 2):
                        # lch1 qt (needed from the merged phase), one m-tile
                        # per V chunk so the pair-1 ST flow never starves
                        q_pass_b(1, q - 1)
                    for half in range(2):
                        v_ps = [psv.tile([P, M], f32, tag="vv",
                                         name=f"vps{q}_{half}_{i}")
                                for i in range(2)]
                        for ck in range(CK):
                            for st2 in range(2):
                                st4 = half * 2 + st2
                                nc.tensor.matmul(
                                    v_ps[st2][:],
                                    xv_t[q][:, ck, st4 * P:(st4 + 1) * P],
                                    wv_sb[:, ck, :],
                                    start=(ck == 0), stop=(ck == CK - 1))
                            if ck in (3, 7) and b_jobs:
                                pair, st = b_jobs.pop(0)
                                pt0[(pair, st)] = st_step(0, pair, st)
                        for st2 in range(2):
                            st = q * 4 + half * 2 + st2
                            nc.vector.tensor_copy(
                                vones[:, st, :, 0:D],
                                v_ps[st2][:].rearrange("p (h d) -> p h d", h=HPC))
                for pair, st in b_jobs:
                    pt0[(pair, st)] = st_step(0, pair, st)

            # ====== Merged phase: per slot: O(lch0) + O(lch1) + ST(lch1).
            # pair1's O accumulations lag (3 and 5 slots) so the pair0 norms
            # have PE cover before pair1's matmuls alias pair0's PSUM banks.
            o_ps0 = {}   # pair -> [o_ps hh0, hh1] for lch0
            o_ps1 = {}
            # right side: independent pool stack, so pst (left) can close
            # before the norms finish and ps_wo reuses pst's banks.
            with tc.tile_pool(name="ps_o", bufs=4, space="PSUM",
                              side="right") as pso:
                def o_tiles(lch, pair):
                    return [pso.tile([D + 1, LCH], f32, tag="oo",
                                     name=f"ops{lch}_{pair}_{h}")
                            for h in range(2)]

                o_ps0[0] = o_tiles(0, 0)
                o_ps1[0] = o_tiles(1, 0)
                for s in range(NST):
                    # ST mid-slot: its kt weight loads hide behind O(lch0)'s
                    # streams.  (pt pool rotation is 4 slots deep, so the ST
                    # write only depends on slot s-4's O reads.)
                    o_step(o_ps0[0], s, 0, pt0.pop((0, s)))
                    pt1[(0, s)] = st_step(1, 0, s)
                    if s >= 2:
                        o_step(o_ps1[0], s - 2, 0, pt1.pop((0, s - 2)))
                norm_lch(0, 0, o_ps0[0])
                o_ps0[1] = o_tiles(0, 1)
                o_ps1[1] = o_tiles(1, 1)
                for s in range(NST):
                    if s < 2:
                        o_step(o_ps1[0], 14 + s, 0, pt1.pop((0, 14 + s)))
                    if s == 1:
                        norm_lch(0, 1, o_ps1[0])
                    if s >= 3:
                        o_step(o_ps0[1], s - 3, 1, pt0.pop((1, s - 3)))
                    pt1[(1, s)] = st_step(1, 1, s)
                    if s >= 5:
                        o_step(o_ps1[1], s - 5, 1, pt1.pop((1, s - 5)))
                # drains + tail norms (Wo-lch0 unblocks after norm(1,0))
                for s in range(13, 16):
                    o_step(o_ps0[1], s, 1, pt0.pop((1, s)))
                norm_lch(1, 0, o_ps0[1], tail=True)
                for s in range(11, 16):
                    o_step(o_ps1[1], s, 1, pt1.pop((1, s)))
                norm_lch(1, 1, o_ps1[1], tail=True)

                pst_cm.__exit__(None, None, None)

                # ======= Wo tail: lch0 jobs first (unblocked earlier) =======
                with tc.tile_pool(name="ps_wo", bufs=4, space="PSUM") as pswo:
                    ncast = 0
                    for lt in range(8):
                        for nch in range(2):
                            wo_step(pswo, lt, nch,
                                    "scalar" if ncast % 2 == 0 else "vector")
                            ncast += 1

    nc.compile()
    return nc


def _get_nc():
    if "nc" not in _cache:
        _cache["nc"] = _build()
    return _cache["nc"]


def _make_in_maps(inputs):
    import ml_dtypes

    bf16 = ml_dtypes.bfloat16
    query = np.asarray(inputs["query"], dtype=np.float32)
    key = np.asarray(inputs["key"], dtype=np.float32)
    value = np.asarray(inputs["value"], dtype=np.float32)
    Wq = np.asarray(inputs["Wq"], dtype=np.float32)
    Wk = np.asarray(inputs["Wk"], dtype=np.float32)
    Wv = np.asarray(inputs["Wv"], dtype=np.float32)
    Wo = np.asarray(inputs["Wo"], dtype=np.float32)

    def tile_w(W):
        # [C, M] -> [P, CK, M]
        return np.ascontiguousarray(
            W.reshape(CK, P, M).transpose(1, 0, 2)).astype(bf16)

    def tile_wo(W):
        # [M, C] -> [P, 2, C]
        return np.ascontiguousarray(
            W.reshape(2, P, C).transpose(1, 0, 2)).astype(bf16)

    def tile_xq(x):
        # x [L, C]; xT [C, L] -> [P, 2, CK, LCH]
        return np.ascontiguousarray(
            x.T.reshape(CK, P, 2, LCH).transpose(1, 2, 0, 3)).astype(bf16)

    def tile_xs(x, widths, offs):
        # x [S, C]; xT [C, S] -> [P, CK*S], slice-major, each slice
        # [P, CK, w] flattened (per-partition contiguous)
        xT = x.T
        parts = [
            np.ascontiguousarray(
                xT[:, o:o + w].reshape(CK, P, w).transpose(1, 0, 2)
            ).reshape(P, CK * w)
            for o, w in zip(offs, widths)
        ]
        return np.ascontiguousarray(np.concatenate(parts, axis=1)).astype(bf16)

    XV_W = [512] * 4
    XV_OFF = [0, 512, 1024, 1536]
    qt_ = [tile_xq(query[b]) for b in range(B)]
    kt_ = [tile_xs(key[b], KSL, KSL_OFF) for b in range(B)]
    vt_ = [tile_xs(value[b], XV_W, XV_OFF) for b in range(B)]
    wq_s = [tile_w(Wq[:, g * M:(g + 1) * M]) for g in range(4)]
    wk_s = [tile_w(Wk[:, g * M:(g + 1) * M]) for g in range(4)]
    wv_s = [tile_w(Wv[:, g * M:(g + 1) * M]) for g in range(4)]
    wo_s = [tile_wo(Wo[g * M:(g + 1) * M, :]) for g in range(4)]

    in_maps = []
    for core in range(NCORES):
        b, g = core // 4, core % 4
        in_maps.append({
            "xqT": qt_[b], "xkT": kt_[b], "xvT": vt_[b],
            "wq": wq_s[g], "wk": wk_s[g], "wv": wv_s[g], "wo": wo_s[g],
        })
    return in_maps


def kernel(query, key, value, Wq, Wk, Wv, Wo, bo):
    from concourse.bass_utils import run_bass_kernel_spmd

    nc = _get_nc()
    bo = np.asarray(bo, dtype=np.float32)
    in_maps = _make_in_maps(dict(query=query, key=key, value=value,
                                 Wq=Wq, Wk=Wk, Wv=Wv, Wo=Wo))

    res = run_bass_kernel_spmd(nc, in_maps, core_ids=list(range(NCORES)))

    out = np.zeros((B, L, C), dtype=np.float32)
    for core in range(NCORES):
        b = core // 4
        out[b] += np.asarray(res.results[core]["outp"], dtype=np.float32)
    out += bo[None, None, :]
    return out
